# revision 68
# baseline (speedup 1.0000x reference)
"""AdaptiveLSTMCellWithRes on 8 TRN2 NeuronCores.

Data-parallel over batch (1024 rows/core), weights replicated.
All on-chip compute happens in transposed-activation space [feat, batch].
Mixed precision:
  - i, s, c_hat, a1 matmuls run fp8(e4m3) with DoubleRow perf mode
    (2 k-tiles per PE pass, 2x the bf16 rate). Weights are pre-scaled
    by 64 on host (0.02-std values would land subnormal in e4m3);
    the 1/64 folds into the PSUM-evicting activation's scale.
  - f, o, residual chain and a2 (the error-critical terms) run bf16.
  - PSUM, biases and the elementwise combine stay fp32; h/c outputs
    are written bf16 (well inside the error budget, halves store DMA).
DMA dispatch costs ~0.6us per dma_start on the issuing engine, so
activations/c_prev/outputs ride the Scalar HWDGE queue in a few big
host-packed transfers while weight slabs stream on the Sync queue.
"""

import sys

if "/opt/trn_rl_repo" not in sys.path:
    sys.path.insert(0, "/opt/trn_rl_repo")

import numpy as np
import ml_dtypes

P = 128
B = 8192          # global batch
NCORES = 8
BL = B // NCORES  # batch per core (1024)
D = 1024          # feature dim
K2 = 2048         # concat(x, h) contraction
JC = D // P       # 8 output-feature tiles
KC2 = K2 // P     # 16 k-chunks for gates/a1
KC1 = D // P      # 8 k-chunks for residual/a2
TC2 = KC2 // 2    # 8 fp8 double-row pair steps
NH = BL // 2      # moving free dim per matmul (512)
WS = 64.0         # fp8 weight pre-scale
QK = 4            # k-chunks per packed activation quarter-tile

E4NP = ml_dtypes.float8_e4m3
BFNP = ml_dtypes.bfloat16

_CACHE = {}


def _build(dbg=False):
    import concourse.bass as bass  # noqa: F401
    from concourse import bacc, mybir
    import concourse.tile as tile

    F32 = mybir.dt.float32
    BF16 = mybir.dt.bfloat16
    FP8 = mybir.dt.float8e4
    AF = mybir.ActivationFunctionType
    DR = mybir.MatmulPerfMode.DoubleRow

    nc = bacc.Bacc()

    # fp8 gates (c, s, i, a1): pack[g, j, p, kk, m] = e4m3(WS * Wg[j*128+m, kk*128+p])
    w8 = nc.declare_dram_parameter("w8", [4, JC, P, KC2, P], FP8, isOutput=False)
    # fp8 r1 weight, same packing with K=D
    w8r1 = nc.declare_dram_parameter("w8r1", [JC, P, KC1, P], FP8,
                                     isOutput=False)
    # bf16 gates (f, o): pack[g, j, p, k*128+m] = W[j*128+m, k*128+p]
    wf = nc.declare_dram_parameter("wf", [2, JC, P, K2], BF16, isOutput=False)
    # residual weights (r1, r2, r3): packed [3, JC, P, D] bf16
    wr = nc.declare_dram_parameter("wr", [3, JC, P, D], BF16, isOutput=False)
    # a2 weight: [P, KC1] bf16 with a2p[p, k] = a2_w[0, k*128+p]
    a2p = nc.declare_dram_parameter("a2p", [P, KC1], BF16, isOutput=False)
    # biases: [P, 10*JC]; col v*JC+j holds vec_v[j*128:(j+1)*128]
    # v: 0..4 = combined gate biases (i,f,o,c,s), 5=a1_b, 6=r1_b, 7=r2_b,
    # 8=r3_b, 9=a2_b (replicated)
    biasp = nc.declare_dram_parameter("biasp", [P, 10 * JC], F32, isOutput=False)
    # activations pre-swizzled into quarter tiles: [q, p, kk, n] =
    # act[(q*QK+kk)*128+p, n]; q 0..1 = x^T, 2..3 = h^T
    xh16s = nc.declare_dram_parameter("xh16s", [4, P, QK, BL], BF16,
                                      isOutput=False)
    xh8s = nc.declare_dram_parameter("xh8s", [4, P, QK, BL], FP8,
                                     isOutput=False)
    cTs = nc.declare_dram_parameter("cTs", [2, P, QK, BL], BF16, isOutput=False)
    # out[d, 0, n] = h_t^T, out[d, 1, n] = c_t^T (bf16)
    out = nc.declare_dram_parameter("out", [D, 2, BL], BF16, isOutput=True)

    alpha_dram = nc.dram_tensor("alpha_dram", [1, BL], F32)

    if dbg:
        dbg8 = nc.declare_dram_parameter("dbg8", [P, 2, BL], FP8, isOutput=True)
        dbga1 = nc.declare_dram_parameter("dbga1", [P, NH], BF16, isOutput=True)
        dbgal = nc.declare_dram_parameter("dbgal", [P, BL], F32, isOutput=True)
        dbgr1 = nc.declare_dram_parameter("dbgr1", [P, BL], BF16, isOutput=True)
        dbgr2 = nc.declare_dram_parameter("dbgr2", [P, BL], BF16, isOutput=True)
        dbgg = nc.declare_dram_parameter("dbgg", [6, P, BL], F32, isOutput=True)

    with tile.TileContext(nc) as tc:
        with (
            tc.tile_pool(name="consts", bufs=1) as consts,
            tc.tile_pool(name="xh8", bufs=1) as xh8_pool,
            tc.tile_pool(name="xh16", bufs=1) as xh16_pool,
            tc.tile_pool(name="cpre", bufs=1) as cp_pool,
            tc.tile_pool(name="w8p", bufs=4) as w8_pool,
            tc.tile_pool(name="wfp", bufs=4) as wf_pool,
            tc.tile_pool(name="a1s", bufs=4) as a1_pool,
            tc.tile_pool(name="r1", bufs=1) as r1_pool,
            tc.tile_pool(name="r2", bufs=1) as r2_pool,
            tc.tile_pool(name="gates", bufs=1) as g_pool,
            tc.tile_pool(name="ew", bufs=2) as ew_pool,
            tc.tile_pool(name="psum", bufs=3, space="PSUM") as psum_pool,
            tc.tile_pool(name="psum_a2", bufs=1, space="PSUM") as psum_a2_pool,
        ):
            bias_sb = consts.tile([P, 10 * JC], F32, name="bias_sb")
            a2_sb = consts.tile([P, KC1], BF16, name="a2_sb")

            def bias_ap(v, j):
                return bias_sb[:, v * JC + j: v * JC + j + 1]

            def load_w8(g, j):
                wt = w8_pool.tile([P, KC2, P], FP8, tag="w8", name=f"w8_{g}_{j}")
                nc.sync.dma_start(out=wt[:], in_=w8[g, j])
                return wt

            # ---- prefix. Scalar queue: ONLY the fp8 xh stream (a1/r1 are
            # the first phases) — DMA dispatches share the ScalarE
            # instruction stream with PSUM evictions, so it must stay
            # near-empty. Sync: consts then in-loop weight slabs. GpSimd
            # (idle otherwise): all bulk loads needed only in phase B.
            nc.sync.dma_start(out=bias_sb[:], in_=biasp[:, :])
            nc.sync.dma_start(out=a2_sb[:], in_=a2p[:, :])
            xh8q = []
            for q in range(4):
                tl = xh8_pool.tile([P, QK, BL], FP8, tag=f"x8{q}", name=f"x8{q}")
                nc.scalar.dma_start(out=tl[:], in_=xh8s[q])
                xh8q.append(tl)
            xh16q = [None] * 4
            cpq = []

            def load_bulk_xh16(dep_ap):
                # phase-B bulk loads on the otherwise-idle gpsimd queue.
                # The scheduler orders DMAs by dependency, not program
                # order, so a tiny write sourced from an early phase-A
                # product (overwritten by the DMA) holds these transfers
                # back until the critical fp8 prefix stream has landed.
                for q in (0, 1, 2, 3):
                    tl = xh16_pool.tile([P, QK, BL], BF16, tag=f"x{q}",
                                        name=f"x{q}")
                    nc.scalar.copy(tl[0:1, 0, 0:1], dep_ap)
                    nc.gpsimd.dma_start(out=tl[:], in_=xh16s[q])
                    xh16q[q] = tl

            def load_bulk_cp(dep_ap):
                for q in range(2):
                    tl = cp_pool.tile([P, QK, BL], BF16, tag=f"cp{q}",
                                      name=f"cp{q}")
                    nc.scalar.copy(tl[0:1, 0, 0:1], dep_ap)
                    nc.gpsimd.dma_start(out=tl[:], in_=cTs[q])
                    cpq.append(tl)

            if dbg:
                nc.scalar.dma_start(out=dbg8[:, :, :], in_=xh8q[0][:, 0:2, :])

            def rhs_xh8(t, mv):
                q, kk = divmod(2 * t, QK)
                return xh8q[q][:, kk:kk + 2, mv]

            def rhs_xh16(k, mv):
                q, kk = divmod(k, QK)
                return xh16q[q][:, kk:kk + 1, mv]

            def load_wg16(g, j):
                wt = wf_pool.tile([P, K2], BF16, tag="wg", name=f"wg_{g}_{j}",
                                  bufs=3)
                nc.sync.dma_start(out=wt[:], in_=wf[g, j])
                return wt

            def load_wr16(ri, j):
                wt = wf_pool.tile([P, D], BF16, tag="wr", name=f"wr_{ri}_{j}",
                                  bufs=6)
                nc.sync.dma_start(out=wt[:], in_=wr[ri, j])
                return wt

            def ps_pair(nm):
                return [psum_pool.tile([P, NH], F32, tag="ps0", name=f"{nm}0"),
                        psum_pool.tile([P, NH], F32, tag="ps1", name=f"{nm}1")]

            def mm8(ps2, wt, tc=TC2, t0=0):
                # fp8 DoubleRow over tc k-pairs starting at pair t0; each
                # stationary pair feeds both batch halves
                for t in range(tc):
                    for bh in range(2):
                        mv = slice(bh * NH, (bh + 1) * NH)
                        nc.tensor.matmul(
                            ps2[bh][:], wt[:, 2 * t:2 * t + 2, :],
                            rhs_xh8(t0 + t, mv),
                            start=(t == 0), stop=(t == tc - 1),
                            perf_mode=DR)

            def mm16(ps2, wt, rhs, kc, koff=0):
                # k outer / bh inner: each stationary tile feeds 2 matmuls
                for k in range(kc):
                    for bh in range(2):
                        mv = slice(bh * NH, (bh + 1) * NH)
                        nc.tensor.matmul(
                            ps2[bh][:], wt[:, k * P:(k + 1) * P],
                            rhs(koff + k, mv),
                            start=(k == 0), stop=(k == kc - 1))

            # ---- phase A1: a1 (fp8), deferred tiny a2 matmuls ----
            ps_a2 = [psum_a2_pool.tile([1, NH], F32, tag="a20", name="psa20"),
                     psum_a2_pool.tile([1, NH], F32, tag="a21", name="psa21")]
            pend = []

            def flush_a2():
                jq, pair = pend.pop(0)
                for bh in range(2):
                    nc.tensor.matmul(ps_a2[bh][:], a2_sb[:, jq:jq + 1],
                                     pair[bh][:], start=(jq == 0),
                                     stop=(jq == JC - 1))

            for j in range(JC):
                wt = load_w8(3, j)
                ps2 = ps_pair("ps_a1_")
                mm8(ps2, wt)
                pair = []
                for bh in range(2):
                    a1b = a1_pool.tile([P, NH], BF16, tag="a1", name="a1b")
                    nc.scalar.activation(a1b[:], ps2[bh][:], AF.Relu,
                                         bias=bias_ap(5, j), scale=1.0 / WS)
                    pair.append(a1b)
                    if dbg and j == 0 and bh == 0:
                        nc.scalar.dma_start(out=dbga1[:, :], in_=a1b[:])
                pend.append((j, pair))
                # defer the tiny a2 matmuls one j so PE never waits on ScalarE
                if len(pend) == 2:
                    flush_a2()
            while pend:
                flush_a2()

            # alpha = sigmoid(a2 @ a1relu + a2_b): [1, BL]; broadcast via
            # DRAM roundtrip that hides under the r1/r2 phases
            for bh in range(2):
                asb = a1_pool.tile([1, NH], F32, tag="a1", name="alpha_sb")
                nc.scalar.activation(asb[:], ps_a2[bh][:], AF.Sigmoid,
                                     bias=bias_sb[0:1, 9 * JC: 9 * JC + 1])
                nc.sync.dma_start(
                    out=alpha_dram[0:1, bh * NH:(bh + 1) * NH], in_=asb[:])
            alpha_rep = consts.tile([P, BL], F32, name="alpha_rep")
            nc.gpsimd.dma_start(
                out=alpha_rep[:], in_=alpha_dram[0:1, :].broadcast_to([P, BL]))
            if dbg:
                nc.scalar.dma_start(out=dbgal[:, :], in_=alpha_rep[:])

            # ---- phase A2: r1 = relu(h @ r1_w.T + b) in fp8 (moving = the
            # resident fp8 h-half; the r1->r2->r3 chain keeps r2/r3 bf16) ----
            r1 = []
            for j in range(JC):
                wt = w8_pool.tile([P, KC1, P], FP8, tag="w8r", bufs=3,
                                  name=f"w8r1_{j}")
                nc.sync.dma_start(out=wt[:], in_=w8r1[j])
                t_ = r1_pool.tile([P, BL], BF16, tag=f"r1_{j}", name=f"r1_{j}")
                ps2 = ps_pair("ps_r1_")
                mm8(ps2, wt, tc=KC1 // 2, t0=TC2 // 2)
                for bh in range(2):
                    nc.scalar.activation(t_[:, bh * NH:(bh + 1) * NH], ps2[bh][:],
                                         AF.Relu, bias=bias_ap(6, j),
                                         scale=1.0 / WS)
                r1.append(t_)
                if j == 0:
                    load_bulk_xh16(t_[0:1, 0:1])
                    load_bulk_cp(t_[0:1, 0:1])
            if dbg:
                nc.scalar.dma_start(out=dbgr1[:, :], in_=r1[0][:])

            def rhs_r1(k, mv):
                return r1[k][:, mv]

            def rhs_r2(k, mv):
                return r2[k][:, mv]

            # ---- phase A3: r2 = relu(r1 @ r2_w.T + b) in bf16 ----
            r2 = []
            for j in range(JC):
                wt = load_wr16(1, j)
                t_ = r2_pool.tile([P, BL], BF16, tag=f"r2_{j}", name=f"r2_{j}")
                ps2 = ps_pair("ps_r2_")
                mm16(ps2, wt, rhs_r1, KC1)
                for bh in range(2):
                    nc.scalar.activation(t_[:, bh * NH:(bh + 1) * NH], ps2[bh][:],
                                         AF.Relu, bias=bias_ap(7, j))
                r2.append(t_)
            if dbg:
                nc.scalar.dma_start(out=dbgr2[:, :], in_=r2[0][:])

            # ---- phase B: gates + r3 + combine, per feature tile j.
            # Order c,s,i (fp8), o, f (bf16), r3: the elementwise chain runs
            # while later matmuls stream; o comes before f/r3 so only the
            # short r3-evict -> add -> tanh -> mul chain trails the last MM.
            GATE8 = {"c": (0, 3, AF.Tanh), "s": (1, 4, AF.Sigmoid),
                     "i": (2, 0, AF.Sigmoid)}

            def gate8(key, j):
                gi, v, fn = GATE8[key]
                wt = load_w8(gi, j)
                t_ = g_pool.tile([P, BL], F32, tag=f"g8{key}", name=f"g8{key}")
                ps2 = ps_pair("ps_g8")
                mm8(ps2, wt)
                for bh in range(2):
                    nc.scalar.activation(t_[:, bh * NH:(bh + 1) * NH],
                                         ps2[bh][:], fn, bias=bias_ap(v, j),
                                         scale=1.0 / WS)
                return t_

            def gate16(gi, v, j):
                wt = load_wg16(gi, j)
                t_ = g_pool.tile([P, BL], F32, tag=f"g16{gi}", name=f"g16{gi}")
                ps2 = ps_pair("ps_g16")
                mm16(ps2, wt, rhs_xh16, KC2)
                for bh in range(2):
                    nc.scalar.activation(t_[:, bh * NH:(bh + 1) * NH],
                                         ps2[bh][:], AF.Sigmoid,
                                         bias=bias_ap(v, j))
                return t_

            for j in range(JC):
                ch = gate8("c", j)
                st = gate8("s", j)
                it = gate8("i", j)

                t1s = []
                for bh in range(2):
                    mv = slice(bh * NH, (bh + 1) * NH)
                    t1 = ew_pool.tile([P, NH], F32, tag=f"t1{bh}", name="t1")
                    nc.vector.tensor_mul(t1[:], it[:, mv], ch[:, mv])
                    nc.vector.tensor_mul(t1[:], t1[:], st[:, mv])
                    nc.vector.tensor_mul(t1[:], t1[:], alpha_rep[:, mv])
                    t1s.append(t1)

                ot = gate16(1, 2, j)

                ft = gate16(0, 1, j)
                for bh in range(2):
                    mv = slice(bh * NH, (bh + 1) * NH)
                    qq, kk = divmod(j, QK)
                    t2 = ew_pool.tile([P, NH], F32, tag=f"t2{bh}", name="t2",
                                      bufs=1)
                    nc.vector.tensor_mul(t2[:], ft[:, mv],
                                         cpq[qq][:, kk, mv])
                    nc.vector.tensor_add(t1s[bh][:], t1s[bh][:], t2[:])

                wt = load_wr16(2, j)
                ps2 = ps_pair("ps_r3_")
                mm16(ps2, wt, rhs_r2, KC1)
                if dbg and j == 0:
                    r3d = g_pool.tile([P, BL], F32, tag="r3d", name="r3d")
                    for bh in range(2):
                        nc.scalar.activation(
                            r3d[:, bh * NH:(bh + 1) * NH], ps2[bh][:],
                            AF.Identity, bias=bias_ap(8, j))
                    for gi, gt in enumerate([ch, st, it, ot, ft, r3d]):
                        nc.scalar.dma_start(out=dbgg[gi], in_=gt[:])
                for bh in range(2):
                    mv = slice(bh * NH, (bh + 1) * NH)
                    # stage[:, 0, :] = h, stage[:, 1, :] = c -> single store.
                    # c = (r3_psum + r3_bias) + t1 in one DVE op straight
                    # from PSUM: no ScalarE eviction on the r3 path.
                    stg = ew_pool.tile([P, 2, NH], BF16, tag=f"st{bh}",
                                       name="stg")
                    nc.vector.scalar_tensor_tensor(
                        stg[:, 1, :], ps2[bh][:], bias_ap(8, j), t1s[bh][:],
                        mybir.AluOpType.add, mybir.AluOpType.add)
                    th = ew_pool.tile([P, NH], F32, tag=f"th{bh}", name="th",
                                      bufs=1)
                    nc.scalar.activation(th[:], stg[:, 1, :], AF.Tanh)
                    nc.vector.tensor_mul(stg[:, 0, :], ot[:, mv], th[:])
                    nc.scalar.dma_start(out=out[j * P:(j + 1) * P, :, mv],
                                        in_=stg[:])

    nc.finalize()
    return nc


def _pack_w(W, kdim):
    # pack[j, p, k*128+m] = W[j*128+m, k*128+p]
    kc = kdim // P
    return np.ascontiguousarray(
        np.asarray(W, np.float32).reshape(JC, P, kc, P)
        .transpose(0, 3, 2, 1).reshape(JC, P, kc * P))


def _pack_act(aT, nq, qk=QK):
    # aT: [nq*qk*P, BL] -> [nq, P, qk, BL] with [q, p, kk, n] = aT[(q*qk+kk)*P+p, n]
    return np.ascontiguousarray(
        aT.reshape(nq, qk, P, BL).transpose(0, 2, 1, 3))


def _prepare(inputs):
    f = lambda name: np.asarray(inputs[name], dtype=np.float32)

    def comb(g):
        u = "U" + g[1]
        return np.concatenate([f(g + "_w"), f(u + "_w")], axis=1)

    # fp8 gates: c, s, i, a1 (order matches in-kernel GATE8/a1 indices)
    w8 = np.stack([
        _pack_w(comb("Wc") * WS, K2),
        _pack_w(comb("Ws") * WS, K2),
        _pack_w(comb("Wi") * WS, K2),
        _pack_w(f("a1_w") * WS, K2),
    ]).astype(E4NP).reshape(4, JC, P, KC2, P)
    w8r1 = (_pack_w(f("r1_w") * WS, D).astype(E4NP)
            .reshape(JC, P, KC1, P))
    # bf16 gates: f, o
    wf_ = np.stack([_pack_w(comb("Wf"), K2),
                    _pack_w(comb("Wo"), K2)]).astype(BFNP)
    wr_ = np.stack(
        [_pack_w(f(n + "_w"), D) for n in ("r1", "r2", "r3")]).astype(BFNP)
    a2p = np.ascontiguousarray(f("a2_w").reshape(KC1, P).T).astype(BFNP)

    bias_vecs = []
    for g in ("Wi", "Wf", "Wo", "Wc", "Ws"):
        u = "U" + g[1]
        bias_vecs.append(f(g + "_b") + f(u + "_b"))
    bias_vecs += [f("a1_b"), f("r1_b"), f("r2_b"), f("r3_b"),
                  np.full(D, f("a2_b")[0], np.float32)]
    # biasp[p, v*JC + j] = vec_v[j*128 + p]
    biasp = np.ascontiguousarray(
        np.stack(bias_vecs).reshape(10, JC, P).transpose(2, 0, 1)
        .reshape(P, 10 * JC))

    x, h, c = f("x"), f("h_prev"), f("c_prev")
    shared = {"w8": w8, "w8r1": w8r1, "wf": wf_, "wr": wr_, "a2p": a2p,
              "biasp": biasp}
    in_maps = []
    for core in range(NCORES):
        sl = slice(core * BL, (core + 1) * BL)
        xhT = np.ascontiguousarray(
            np.concatenate([x[sl].T, h[sl].T], axis=0))  # [K2, BL]
        in_maps.append({**shared,
                        "xh16s": _pack_act(xhT.astype(BFNP), 4),
                        "xh8s": _pack_act(xhT.astype(E4NP), 4),
                        "cTs": _pack_act(
                            np.ascontiguousarray(c[sl].T).astype(BFNP), 2)})
    return in_maps


def _run(inputs, trace=False):
    from concourse.bass_utils import run_bass_kernel_spmd

    if "nc" not in _CACHE:
        _CACHE["nc"] = _build()
    nc = _CACHE["nc"]
    in_maps = _prepare(inputs)
    res = run_bass_kernel_spmd(nc, in_maps, core_ids=list(range(NCORES)),
                               trace=trace)
    h = np.empty((B, D), np.float32)
    c = np.empty((B, D), np.float32)
    for core in range(NCORES):
        o = res.results[core]["out"]  # [D, 2, BL] bf16
        sl = slice(core * BL, (core + 1) * BL)
        h[sl] = o[:, 0].T.astype(np.float32)
        c[sl] = o[:, 1].T.astype(np.float32)
    return (h, c), res


def kernel(**inputs):
    (h, c), _ = _run(inputs, trace=False)
    return (h, c)


# revision 69
# speedup vs baseline: 1.1840x; 1.1840x over previous
"""AdaptiveLSTMCellWithRes on 8 TRN2 NeuronCores.

Data-parallel over batch (1024 rows/core), weights replicated.
All on-chip compute happens in transposed-activation space [feat, batch].
Mixed precision:
  - i, s, c_hat, a1 matmuls run fp8(e4m3) with DoubleRow perf mode
    (2 k-tiles per PE pass, 2x the bf16 rate). Weights are pre-scaled
    by 64 on host (0.02-std values would land subnormal in e4m3);
    the 1/64 folds into the PSUM-evicting activation's scale.
  - f, o, residual chain and a2 (the error-critical terms) run bf16.
  - PSUM, biases and the elementwise combine stay fp32; h/c outputs
    are written bf16 (well inside the error budget, halves store DMA).
DMA dispatch costs ~0.6us per dma_start on the issuing engine, so
activations/c_prev/outputs ride the Scalar HWDGE queue in a few big
host-packed transfers while weight slabs stream on the Sync queue.
"""

import sys

if "/opt/trn_rl_repo" not in sys.path:
    sys.path.insert(0, "/opt/trn_rl_repo")

import numpy as np
import ml_dtypes

P = 128
B = 8192          # global batch
NCORES = 8
BL = B // NCORES  # batch per core (1024)
D = 1024          # feature dim
K2 = 2048         # concat(x, h) contraction
JC = D // P       # 8 output-feature tiles
KC2 = K2 // P     # 16 k-chunks for gates/a1
KC1 = D // P      # 8 k-chunks for residual/a2
TC2 = KC2 // 2    # 8 fp8 double-row pair steps
NH = BL // 2      # moving free dim per matmul (512)
WS = 64.0         # fp8 weight pre-scale
QK = 4            # k-chunks per packed activation quarter-tile

E4NP = ml_dtypes.float8_e4m3
BFNP = ml_dtypes.bfloat16

_CACHE = {}


def _build(dbg=False):
    import concourse.bass as bass  # noqa: F401
    from concourse import bacc, mybir
    import concourse.tile as tile

    F32 = mybir.dt.float32
    BF16 = mybir.dt.bfloat16
    FP8 = mybir.dt.float8e4
    AF = mybir.ActivationFunctionType
    DR = mybir.MatmulPerfMode.DoubleRow

    nc = bacc.Bacc()

    # fp8 gates (c, s, i, a1): pack[g, j, p, kk, m] = e4m3(WS * Wg[j*128+m, kk*128+p])
    w8 = nc.declare_dram_parameter("w8", [4, JC, P, KC2, P], FP8, isOutput=False)
    # fp8 r1 weight, same packing with K=D
    w8r1 = nc.declare_dram_parameter("w8r1", [JC, P, KC1, P], FP8,
                                     isOutput=False)
    # bf16 gates (f, o): pack[g, j, p, k*128+m] = W[j*128+m, k*128+p]
    wf = nc.declare_dram_parameter("wf", [2, JC, P, K2], BF16, isOutput=False)
    # residual weights (r1, r2, r3): packed [3, JC, P, D] bf16
    wr = nc.declare_dram_parameter("wr", [3, JC, P, D], BF16, isOutput=False)
    # a2 weight: [P, KC1] bf16 with a2p[p, k] = a2_w[0, k*128+p]
    a2p = nc.declare_dram_parameter("a2p", [P, KC1], BF16, isOutput=False)
    # biases: [P, 10*JC]; col v*JC+j holds vec_v[j*128:(j+1)*128]
    # v: 0..4 = combined gate biases (i,f,o,c,s), 5=a1_b, 6=r1_b, 7=r2_b,
    # 8=r3_b, 9=a2_b (replicated)
    biasp = nc.declare_dram_parameter("biasp", [P, 10 * JC], F32, isOutput=False)
    # activations pre-swizzled into quarter tiles: [q, p, kk, n] =
    # act[(q*QK+kk)*128+p, n]; q 0..1 = x^T, 2..3 = h^T
    xh16s = nc.declare_dram_parameter("xh16s", [4, P, QK, BL], BF16,
                                      isOutput=False)
    xh8s = nc.declare_dram_parameter("xh8s", [4, P, QK, BL], FP8,
                                     isOutput=False)
    cTs = nc.declare_dram_parameter("cTs", [2, P, QK, BL], BF16, isOutput=False)
    # out[d, 0, n] = h_t^T, out[d, 1, n] = c_t^T (bf16)
    out = nc.declare_dram_parameter("out", [D, 2, BL], BF16, isOutput=True)

    alpha_dram = nc.dram_tensor("alpha_dram", [1, BL], F32)

    if dbg:
        dbg8 = nc.declare_dram_parameter("dbg8", [P, 2, BL], FP8, isOutput=True)
        dbga1 = nc.declare_dram_parameter("dbga1", [P, NH], BF16, isOutput=True)
        dbgal = nc.declare_dram_parameter("dbgal", [P, BL], F32, isOutput=True)
        dbgr1 = nc.declare_dram_parameter("dbgr1", [P, BL], BF16, isOutput=True)
        dbgr2 = nc.declare_dram_parameter("dbgr2", [P, BL], BF16, isOutput=True)
        dbgg = nc.declare_dram_parameter("dbgg", [6, P, BL], F32, isOutput=True)

    with tile.TileContext(nc) as tc:
        with (
            tc.tile_pool(name="consts", bufs=1) as consts,
            tc.tile_pool(name="xh8", bufs=1) as xh8_pool,
            tc.tile_pool(name="xh16", bufs=1) as xh16_pool,
            tc.tile_pool(name="cpre", bufs=1) as cp_pool,
            tc.tile_pool(name="w8p", bufs=6) as w8_pool,
            tc.tile_pool(name="wfp", bufs=4) as wf_pool,
            tc.tile_pool(name="a1s", bufs=4) as a1_pool,
            tc.tile_pool(name="r1", bufs=1) as r1_pool,
            tc.tile_pool(name="r2", bufs=1) as r2_pool,
            tc.tile_pool(name="gates", bufs=1) as g_pool,
            tc.tile_pool(name="ew", bufs=2) as ew_pool,
            tc.tile_pool(name="psum", bufs=3, space="PSUM") as psum_pool,
            tc.tile_pool(name="psum_a2", bufs=1, space="PSUM") as psum_a2_pool,
        ):
            bias_sb = consts.tile([P, 10 * JC], F32, name="bias_sb")
            a2_sb = consts.tile([P, KC1], BF16, name="a2_sb")

            def bias_ap(v, j):
                return bias_sb[:, v * JC + j: v * JC + j + 1]

            def load_w8(g, j):
                wt = w8_pool.tile([P, KC2, P], FP8, tag="w8", name=f"w8_{g}_{j}")
                nc.sync.dma_start(out=wt[:], in_=w8[g, j])
                return wt

            # ---- prefix. Scalar queue: ONLY the fp8 xh stream (a1/r1 are
            # the first phases) — DMA dispatches share the ScalarE
            # instruction stream with PSUM evictions, so it must stay
            # near-empty. Sync: consts then in-loop weight slabs. GpSimd
            # (idle otherwise): all bulk loads needed only in phase B.
            nc.sync.dma_start(out=bias_sb[:], in_=biasp[:, :])
            nc.sync.dma_start(out=a2_sb[:], in_=a2p[:, :])
            xh8q = []
            for q in range(4):
                tl = xh8_pool.tile([P, QK, BL], FP8, tag=f"x8{q}", name=f"x8{q}")
                nc.scalar.dma_start(out=tl[:], in_=xh8s[q])
                xh8q.append(tl)
            xh16q = [None] * 4
            cpq = []

            def load_bulk_xh16(dep_ap):
                # phase-B bulk loads on the otherwise-idle gpsimd queue.
                # The scheduler orders DMAs by dependency, not program
                # order, so a tiny write sourced from an early phase-A
                # product (overwritten by the DMA) holds these transfers
                # back until the critical fp8 prefix stream has landed.
                for q in (0, 1, 2, 3):
                    tl = xh16_pool.tile([P, QK, BL], BF16, tag=f"x{q}",
                                        name=f"x{q}")
                    nc.scalar.copy(tl[0:1, 0, 0:1], dep_ap)
                    nc.gpsimd.dma_start(out=tl[:], in_=xh16s[q])
                    xh16q[q] = tl

            def load_bulk_cp(dep_ap):
                for q in range(2):
                    tl = cp_pool.tile([P, QK, BL], BF16, tag=f"cp{q}",
                                      name=f"cp{q}")
                    nc.scalar.copy(tl[0:1, 0, 0:1], dep_ap)
                    nc.gpsimd.dma_start(out=tl[:], in_=cTs[q])
                    cpq.append(tl)

            if dbg:
                nc.scalar.dma_start(out=dbg8[:, :, :], in_=xh8q[0][:, 0:2, :])

            def rhs_xh8(t, mv):
                q, kk = divmod(2 * t, QK)
                return xh8q[q][:, kk:kk + 2, mv]

            def rhs_xh16(k, mv):
                q, kk = divmod(k, QK)
                return xh16q[q][:, kk:kk + 1, mv]

            def load_wg16(g, j):
                wt = wf_pool.tile([P, K2], BF16, tag="wg", name=f"wg_{g}_{j}",
                                  bufs=3)
                nc.sync.dma_start(out=wt[:], in_=wf[g, j])
                return wt

            def load_wr16(ri, j):
                wt = wf_pool.tile([P, D], BF16, tag="wr", name=f"wr_{ri}_{j}",
                                  bufs=6)
                nc.sync.dma_start(out=wt[:], in_=wr[ri, j])
                return wt

            def ps_pair(nm):
                return [psum_pool.tile([P, NH], F32, tag="ps0", name=f"{nm}0"),
                        psum_pool.tile([P, NH], F32, tag="ps1", name=f"{nm}1")]

            def mm8(ps2, wt, tc=TC2, t0=0):
                # fp8 DoubleRow over tc k-pairs starting at pair t0; each
                # stationary pair feeds both batch halves
                for t in range(tc):
                    for bh in range(2):
                        mv = slice(bh * NH, (bh + 1) * NH)
                        nc.tensor.matmul(
                            ps2[bh][:], wt[:, 2 * t:2 * t + 2, :],
                            rhs_xh8(t0 + t, mv),
                            start=(t == 0), stop=(t == tc - 1),
                            perf_mode=DR)

            def mm16(ps2, wt, rhs, kc, koff=0):
                # k outer / bh inner: each stationary tile feeds 2 matmuls
                for k in range(kc):
                    for bh in range(2):
                        mv = slice(bh * NH, (bh + 1) * NH)
                        nc.tensor.matmul(
                            ps2[bh][:], wt[:, k * P:(k + 1) * P],
                            rhs(koff + k, mv),
                            start=(k == 0), stop=(k == kc - 1))

            # ---- phase A1: a1 (fp8), deferred tiny a2 matmuls ----
            ps_a2 = [psum_a2_pool.tile([1, NH], F32, tag="a20", name="psa20"),
                     psum_a2_pool.tile([1, NH], F32, tag="a21", name="psa21")]
            pend = []

            def flush_a2():
                jq, pair = pend.pop(0)
                for bh in range(2):
                    nc.tensor.matmul(ps_a2[bh][:], a2_sb[:, jq:jq + 1],
                                     pair[bh][:], start=(jq == 0),
                                     stop=(jq == JC - 1))

            for j in range(JC):
                wt = load_w8(3, j)
                ps2 = ps_pair("ps_a1_")
                mm8(ps2, wt)
                pair = []
                for bh in range(2):
                    a1b = a1_pool.tile([P, NH], BF16, tag="a1", name="a1b")
                    nc.scalar.activation(a1b[:], ps2[bh][:], AF.Relu,
                                         bias=bias_ap(5, j), scale=1.0 / WS)
                    pair.append(a1b)
                    if dbg and j == 0 and bh == 0:
                        nc.scalar.dma_start(out=dbga1[:, :], in_=a1b[:])
                pend.append((j, pair))
                # defer the tiny a2 matmuls one j so PE never waits on ScalarE
                if len(pend) == 2:
                    flush_a2()
            while pend:
                flush_a2()

            # alpha = sigmoid(a2 @ a1relu + a2_b): [1, BL]; broadcast via
            # DRAM roundtrip that hides under the r1/r2 phases
            for bh in range(2):
                asb = a1_pool.tile([1, NH], F32, tag="a1", name="alpha_sb")
                nc.scalar.activation(asb[:], ps_a2[bh][:], AF.Sigmoid,
                                     bias=bias_sb[0:1, 9 * JC: 9 * JC + 1])
                nc.sync.dma_start(
                    out=alpha_dram[0:1, bh * NH:(bh + 1) * NH], in_=asb[:])
            alpha_rep = consts.tile([P, BL], F32, name="alpha_rep")
            nc.gpsimd.dma_start(
                out=alpha_rep[:], in_=alpha_dram[0:1, :].broadcast_to([P, BL]))
            if dbg:
                nc.scalar.dma_start(out=dbgal[:, :], in_=alpha_rep[:])

            # ---- phase A2: r1 = relu(h @ r1_w.T + b) in fp8 (moving = the
            # resident fp8 h-half; the r1->r2->r3 chain keeps r2/r3 bf16) ----
            r1 = []
            for j in range(JC):
                wt = w8_pool.tile([P, KC1, P], FP8, tag="w8r", bufs=3,
                                  name=f"w8r1_{j}")
                nc.sync.dma_start(out=wt[:], in_=w8r1[j])
                t_ = r1_pool.tile([P, BL], BF16, tag=f"r1_{j}", name=f"r1_{j}")
                ps2 = ps_pair("ps_r1_")
                mm8(ps2, wt, tc=KC1 // 2, t0=TC2 // 2)
                for bh in range(2):
                    nc.scalar.activation(t_[:, bh * NH:(bh + 1) * NH], ps2[bh][:],
                                         AF.Relu, bias=bias_ap(6, j),
                                         scale=1.0 / WS)
                r1.append(t_)
                if j == 0:
                    load_bulk_xh16(t_[0:1, 0:1])
                    load_bulk_cp(t_[0:1, 0:1])
            if dbg:
                nc.scalar.dma_start(out=dbgr1[:, :], in_=r1[0][:])

            def rhs_r1(k, mv):
                return r1[k][:, mv]

            def rhs_r2(k, mv):
                return r2[k][:, mv]

            # ---- phase A3: r2 = relu(r1 @ r2_w.T + b) in bf16 ----
            r2 = []
            for j in range(JC):
                wt = load_wr16(1, j)
                t_ = r2_pool.tile([P, BL], BF16, tag=f"r2_{j}", name=f"r2_{j}")
                ps2 = ps_pair("ps_r2_")
                mm16(ps2, wt, rhs_r1, KC1)
                for bh in range(2):
                    nc.scalar.activation(t_[:, bh * NH:(bh + 1) * NH], ps2[bh][:],
                                         AF.Relu, bias=bias_ap(7, j))
                r2.append(t_)
            if dbg:
                nc.scalar.dma_start(out=dbgr2[:, :], in_=r2[0][:])

            # ---- phase B: gates + r3 + combine, per feature tile j.
            # Order c,s,i (fp8), o, f (bf16), r3: the elementwise chain runs
            # while later matmuls stream; o comes before f/r3 so only the
            # short r3-evict -> add -> tanh -> mul chain trails the last MM.
            GATE8 = {"c": (0, 3, AF.Tanh), "s": (1, 4, AF.Sigmoid),
                     "i": (2, 0, AF.Sigmoid)}

            def gate8(key, j):
                gi, v, fn = GATE8[key]
                wt = load_w8(gi, j)
                t_ = g_pool.tile([P, BL], F32, tag=f"g8{key}", name=f"g8{key}")
                ps2 = ps_pair("ps_g8")
                mm8(ps2, wt)
                for bh in range(2):
                    nc.scalar.activation(t_[:, bh * NH:(bh + 1) * NH],
                                         ps2[bh][:], fn, bias=bias_ap(v, j),
                                         scale=1.0 / WS)
                return t_

            def gate16(gi, v, j):
                wt = load_wg16(gi, j)
                t_ = g_pool.tile([P, BL], F32, tag=f"g16{gi}", name=f"g16{gi}")
                ps2 = ps_pair("ps_g16")
                mm16(ps2, wt, rhs_xh16, KC2)
                for bh in range(2):
                    nc.scalar.activation(t_[:, bh * NH:(bh + 1) * NH],
                                         ps2[bh][:], AF.Sigmoid,
                                         bias=bias_ap(v, j))
                return t_

            for j in range(JC):
                ch = gate8("c", j)
                st = gate8("s", j)
                it = gate8("i", j)

                t1s = []
                for bh in range(2):
                    mv = slice(bh * NH, (bh + 1) * NH)
                    t1 = ew_pool.tile([P, NH], F32, tag=f"t1{bh}", name="t1")
                    nc.vector.tensor_mul(t1[:], it[:, mv], ch[:, mv])
                    nc.vector.tensor_mul(t1[:], t1[:], st[:, mv])
                    nc.vector.tensor_mul(t1[:], t1[:], alpha_rep[:, mv])
                    t1s.append(t1)

                ot = gate16(1, 2, j)

                ft = gate16(0, 1, j)
                for bh in range(2):
                    mv = slice(bh * NH, (bh + 1) * NH)
                    qq, kk = divmod(j, QK)
                    t2 = ew_pool.tile([P, NH], F32, tag=f"t2{bh}", name="t2",
                                      bufs=1)
                    nc.vector.tensor_mul(t2[:], ft[:, mv],
                                         cpq[qq][:, kk, mv])
                    nc.vector.tensor_add(t1s[bh][:], t1s[bh][:], t2[:])

                wt = load_wr16(2, j)
                ps2 = ps_pair("ps_r3_")
                mm16(ps2, wt, rhs_r2, KC1)
                if dbg and j == 0:
                    r3d = g_pool.tile([P, BL], F32, tag="r3d", name="r3d")
                    for bh in range(2):
                        nc.scalar.activation(
                            r3d[:, bh * NH:(bh + 1) * NH], ps2[bh][:],
                            AF.Identity, bias=bias_ap(8, j))
                    for gi, gt in enumerate([ch, st, it, ot, ft, r3d]):
                        nc.scalar.dma_start(out=dbgg[gi], in_=gt[:])
                for bh in range(2):
                    mv = slice(bh * NH, (bh + 1) * NH)
                    # stage[:, 0, :] = h, stage[:, 1, :] = c -> single store.
                    # c = (r3_psum + r3_bias) + t1 in one DVE op straight
                    # from PSUM: no ScalarE eviction on the r3 path.
                    stg = ew_pool.tile([P, 2, NH], BF16, tag=f"st{bh}",
                                       name="stg")
                    nc.vector.scalar_tensor_tensor(
                        stg[:, 1, :], ps2[bh][:], bias_ap(8, j), t1s[bh][:],
                        mybir.AluOpType.add, mybir.AluOpType.add)
                    th = ew_pool.tile([P, NH], F32, tag=f"th{bh}", name="th",
                                      bufs=1)
                    nc.scalar.activation(th[:], stg[:, 1, :], AF.Tanh)
                    nc.vector.tensor_mul(stg[:, 0, :], ot[:, mv], th[:])
                    nc.scalar.dma_start(out=out[j * P:(j + 1) * P, :, mv],
                                        in_=stg[:])

    nc.finalize()
    return nc


def _pack_w(W, kdim):
    # pack[j, p, k*128+m] = W[j*128+m, k*128+p]
    kc = kdim // P
    return np.ascontiguousarray(
        np.asarray(W, np.float32).reshape(JC, P, kc, P)
        .transpose(0, 3, 2, 1).reshape(JC, P, kc * P))


def _pack_act(aT, nq, qk=QK):
    # aT: [nq*qk*P, BL] -> [nq, P, qk, BL] with [q, p, kk, n] = aT[(q*qk+kk)*P+p, n]
    return np.ascontiguousarray(
        aT.reshape(nq, qk, P, BL).transpose(0, 2, 1, 3))


def _prepare(inputs):
    f = lambda name: np.asarray(inputs[name], dtype=np.float32)

    def comb(g):
        u = "U" + g[1]
        return np.concatenate([f(g + "_w"), f(u + "_w")], axis=1)

    # fp8 gates: c, s, i, a1 (order matches in-kernel GATE8/a1 indices)
    w8 = np.stack([
        _pack_w(comb("Wc") * WS, K2),
        _pack_w(comb("Ws") * WS, K2),
        _pack_w(comb("Wi") * WS, K2),
        _pack_w(f("a1_w") * WS, K2),
    ]).astype(E4NP).reshape(4, JC, P, KC2, P)
    w8r1 = (_pack_w(f("r1_w") * WS, D).astype(E4NP)
            .reshape(JC, P, KC1, P))
    # bf16 gates: f, o
    wf_ = np.stack([_pack_w(comb("Wf"), K2),
                    _pack_w(comb("Wo"), K2)]).astype(BFNP)
    wr_ = np.stack(
        [_pack_w(f(n + "_w"), D) for n in ("r1", "r2", "r3")]).astype(BFNP)
    a2p = np.ascontiguousarray(f("a2_w").reshape(KC1, P).T).astype(BFNP)

    bias_vecs = []
    for g in ("Wi", "Wf", "Wo", "Wc", "Ws"):
        u = "U" + g[1]
        bias_vecs.append(f(g + "_b") + f(u + "_b"))
    bias_vecs += [f("a1_b"), f("r1_b"), f("r2_b"), f("r3_b"),
                  np.full(D, f("a2_b")[0], np.float32)]
    # biasp[p, v*JC + j] = vec_v[j*128 + p]
    biasp = np.ascontiguousarray(
        np.stack(bias_vecs).reshape(10, JC, P).transpose(2, 0, 1)
        .reshape(P, 10 * JC))

    x, h, c = f("x"), f("h_prev"), f("c_prev")
    shared = {"w8": w8, "w8r1": w8r1, "wf": wf_, "wr": wr_, "a2p": a2p,
              "biasp": biasp}
    in_maps = []
    for core in range(NCORES):
        sl = slice(core * BL, (core + 1) * BL)
        xhT = np.ascontiguousarray(
            np.concatenate([x[sl].T, h[sl].T], axis=0))  # [K2, BL]
        in_maps.append({**shared,
                        "xh16s": _pack_act(xhT.astype(BFNP), 4),
                        "xh8s": _pack_act(xhT.astype(E4NP), 4),
                        "cTs": _pack_act(
                            np.ascontiguousarray(c[sl].T).astype(BFNP), 2)})
    return in_maps


def _run(inputs, trace=False):
    from concourse.bass_utils import run_bass_kernel_spmd

    if "nc" not in _CACHE:
        _CACHE["nc"] = _build()
    nc = _CACHE["nc"]
    in_maps = _prepare(inputs)
    res = run_bass_kernel_spmd(nc, in_maps, core_ids=list(range(NCORES)),
                               trace=trace)
    h = np.empty((B, D), np.float32)
    c = np.empty((B, D), np.float32)
    for core in range(NCORES):
        o = res.results[core]["out"]  # [D, 2, BL] bf16
        sl = slice(core * BL, (core + 1) * BL)
        h[sl] = o[:, 0].T.astype(np.float32)
        c[sl] = o[:, 1].T.astype(np.float32)
    return (h, c), res


def kernel(**inputs):
    (h, c), _ = _run(inputs, trace=False)
    return (h, c)


# revision 75
# speedup vs baseline: 1.1935x; 1.0080x over previous
"""AdaptiveLSTMCellWithRes on 8 TRN2 NeuronCores.

Data-parallel over batch (1024 rows/core), weights replicated.
All on-chip compute happens in transposed-activation space [feat, batch].
Mixed precision:
  - i, s, c_hat, a1 matmuls run fp8(e4m3) with DoubleRow perf mode
    (2 k-tiles per PE pass, 2x the bf16 rate). Weights are pre-scaled
    by 64 on host (0.02-std values would land subnormal in e4m3);
    the 1/64 folds into the PSUM-evicting activation's scale.
  - f, o, residual chain and a2 (the error-critical terms) run bf16.
  - PSUM, biases and the elementwise combine stay fp32; h/c outputs
    are written bf16 (well inside the error budget, halves store DMA).
DMA dispatch costs ~0.6us per dma_start on the issuing engine, so
activations/c_prev/outputs ride the Scalar HWDGE queue in a few big
host-packed transfers while weight slabs stream on the Sync queue.
"""

import sys

if "/opt/trn_rl_repo" not in sys.path:
    sys.path.insert(0, "/opt/trn_rl_repo")

import numpy as np
import ml_dtypes

P = 128
B = 8192          # global batch
NCORES = 8
BL = B // NCORES  # batch per core (1024)
D = 1024          # feature dim
K2 = 2048         # concat(x, h) contraction
JC = D // P       # 8 output-feature tiles
KC2 = K2 // P     # 16 k-chunks for gates/a1
KC1 = D // P      # 8 k-chunks for residual/a2
TC2 = KC2 // 2    # 8 fp8 double-row pair steps
NH = BL // 2      # moving free dim per matmul (512)
WS = 64.0         # fp8 weight pre-scale
QK = 4            # k-chunks per packed activation quarter-tile

E4NP = ml_dtypes.float8_e4m3
BFNP = ml_dtypes.bfloat16

_CACHE = {}


def _build(dbg=False):
    import concourse.bass as bass  # noqa: F401
    from concourse import bacc, mybir
    import concourse.tile as tile

    F32 = mybir.dt.float32
    BF16 = mybir.dt.bfloat16
    FP8 = mybir.dt.float8e4
    AF = mybir.ActivationFunctionType
    DR = mybir.MatmulPerfMode.DoubleRow

    nc = bacc.Bacc()

    # fp8 gates (c, s, i, a1): pack[g, j, p, kk, m] = e4m3(WS * Wg[j*128+m, kk*128+p])
    w8 = nc.declare_dram_parameter("w8", [4, JC, P, KC2, P], FP8, isOutput=False)
    # fp8 r1/r2 weights, same packing with K=D
    w8r1 = nc.declare_dram_parameter("w8r1", [JC, P, KC1, P], FP8,
                                     isOutput=False)
    w8r2 = nc.declare_dram_parameter("w8r2", [JC, P, KC1, P], FP8,
                                     isOutput=False)
    # bf16 gates (f, o): pack[g, j, p, k*128+m] = W[j*128+m, k*128+p]
    wf = nc.declare_dram_parameter("wf", [2, JC, P, K2], BF16, isOutput=False)
    # residual weights (r1, r2, r3): packed [3, JC, P, D] bf16
    wr = nc.declare_dram_parameter("wr", [3, JC, P, D], BF16, isOutput=False)
    # a2 weight: [P, KC1] bf16 with a2p[p, k] = a2_w[0, k*128+p]
    a2p = nc.declare_dram_parameter("a2p", [P, KC1], BF16, isOutput=False)
    # biases: [P, 10*JC]; col v*JC+j holds vec_v[j*128:(j+1)*128]
    # v: 0..4 = combined gate biases (i,f,o,c,s), 5=a1_b, 6=r1_b, 7=r2_b,
    # 8=r3_b, 9=a2_b (replicated)
    biasp = nc.declare_dram_parameter("biasp", [P, 10 * JC], F32, isOutput=False)
    # activations pre-swizzled into quarter tiles: [q, p, kk, n] =
    # act[(q*QK+kk)*128+p, n]; q 0..1 = x^T, 2..3 = h^T
    xh16s = nc.declare_dram_parameter("xh16s", [4, P, QK, BL], BF16,
                                      isOutput=False)
    xh8s = nc.declare_dram_parameter("xh8s", [4, P, QK, BL], FP8,
                                     isOutput=False)
    cTs = nc.declare_dram_parameter("cTs", [2, P, QK, BL], BF16, isOutput=False)
    # out[d, 0, n] = h_t^T, out[d, 1, n] = c_t^T (bf16)
    out = nc.declare_dram_parameter("out", [D, 2, BL], BF16, isOutput=True)

    alpha_dram = nc.dram_tensor("alpha_dram", [1, BL], F32)

    if dbg:
        dbg8 = nc.declare_dram_parameter("dbg8", [P, 2, BL], FP8, isOutput=True)
        dbga1 = nc.declare_dram_parameter("dbga1", [P, NH], BF16, isOutput=True)
        dbgal = nc.declare_dram_parameter("dbgal", [P, BL], F32, isOutput=True)
        dbgr1 = nc.declare_dram_parameter("dbgr1", [P, BL], FP8, isOutput=True)
        dbgr2 = nc.declare_dram_parameter("dbgr2", [P, BL], BF16, isOutput=True)
        dbgg = nc.declare_dram_parameter("dbgg", [6, P, BL], F32, isOutput=True)

    with tile.TileContext(nc) as tc:
        with (
            tc.tile_pool(name="consts", bufs=1) as consts,
            tc.tile_pool(name="xh8", bufs=1) as xh8_pool,
            tc.tile_pool(name="xh16", bufs=1) as xh16_pool,
            tc.tile_pool(name="cpre", bufs=1) as cp_pool,
            tc.tile_pool(name="w8p", bufs=6) as w8_pool,
            tc.tile_pool(name="wfp", bufs=4) as wf_pool,
            tc.tile_pool(name="a1s", bufs=4) as a1_pool,
            tc.tile_pool(name="r1", bufs=1) as r1_pool,
            tc.tile_pool(name="r2", bufs=1) as r2_pool,
            tc.tile_pool(name="gates", bufs=1) as g_pool,
            tc.tile_pool(name="ew", bufs=2) as ew_pool,
            tc.tile_pool(name="psum", bufs=3, space="PSUM") as psum_pool,
            tc.tile_pool(name="psum_a2", bufs=1, space="PSUM") as psum_a2_pool,
        ):
            bias_sb = consts.tile([P, 10 * JC], F32, name="bias_sb")
            a2_sb = consts.tile([P, KC1], BF16, name="a2_sb")

            def bias_ap(v, j):
                return bias_sb[:, v * JC + j: v * JC + j + 1]

            def load_w8(g, j):
                wt = w8_pool.tile([P, KC2, P], FP8, tag="w8", name=f"w8_{g}_{j}")
                nc.sync.dma_start(out=wt[:], in_=w8[g, j])
                return wt

            # ---- prefix. Scalar queue: ONLY the fp8 xh stream (a1/r1 are
            # the first phases) — DMA dispatches share the ScalarE
            # instruction stream with PSUM evictions, so it must stay
            # near-empty. Sync: consts then in-loop weight slabs. GpSimd
            # (idle otherwise): all bulk loads needed only in phase B.
            nc.sync.dma_start(out=bias_sb[:], in_=biasp[:, :])
            nc.sync.dma_start(out=a2_sb[:], in_=a2p[:, :])
            xh8q = []
            for q in range(4):
                tl = xh8_pool.tile([P, QK, BL], FP8, tag=f"x8{q}", name=f"x8{q}")
                nc.scalar.dma_start(out=tl[:], in_=xh8s[q])
                xh8q.append(tl)
            xh16q = [None] * 4
            cpq = []

            def load_bulk_xh16(dep_ap):
                # phase-B bulk loads on the otherwise-idle gpsimd queue.
                # The scheduler orders DMAs by dependency, not program
                # order, so a tiny write sourced from an early phase-A
                # product (overwritten by the DMA) holds these transfers
                # back until the critical fp8 prefix stream has landed.
                for q in (0, 1, 2, 3):
                    tl = xh16_pool.tile([P, QK, BL], BF16, tag=f"x{q}",
                                        name=f"x{q}")
                    nc.scalar.copy(tl[0:1, 0, 0:1], dep_ap)
                    nc.gpsimd.dma_start(out=tl[:], in_=xh16s[q])
                    xh16q[q] = tl

            def load_bulk_cp(dep_ap):
                for q in range(2):
                    tl = cp_pool.tile([P, QK, BL], BF16, tag=f"cp{q}",
                                      name=f"cp{q}")
                    nc.scalar.copy(tl[0:1, 0, 0:1], dep_ap)
                    nc.gpsimd.dma_start(out=tl[:], in_=cTs[q])
                    cpq.append(tl)

            if dbg:
                nc.scalar.dma_start(out=dbg8[:, :, :], in_=xh8q[0][:, 0:2, :])

            def rhs_xh8(t, mv):
                q, kk = divmod(2 * t, QK)
                return xh8q[q][:, kk:kk + 2, mv]

            def rhs_xh16(k, mv):
                q, kk = divmod(k, QK)
                return xh16q[q][:, kk:kk + 1, mv]

            def load_wg16(g, j):
                wt = wf_pool.tile([P, K2], BF16, tag="wg", name=f"wg_{g}_{j}",
                                  bufs=3)
                nc.sync.dma_start(out=wt[:], in_=wf[g, j])
                return wt

            def load_wr16(ri, j):
                wt = wf_pool.tile([P, D], BF16, tag="wr", name=f"wr_{ri}_{j}",
                                  bufs=6)
                nc.sync.dma_start(out=wt[:], in_=wr[ri, j])
                return wt

            def ps_pair(nm):
                return [psum_pool.tile([P, NH], F32, tag="ps0", name=f"{nm}0"),
                        psum_pool.tile([P, NH], F32, tag="ps1", name=f"{nm}1")]

            def mm8(ps2, wt, tc=TC2, t0=0):
                # fp8 DoubleRow over tc k-pairs starting at pair t0; each
                # stationary pair feeds both batch halves
                for t in range(tc):
                    for bh in range(2):
                        mv = slice(bh * NH, (bh + 1) * NH)
                        nc.tensor.matmul(
                            ps2[bh][:], wt[:, 2 * t:2 * t + 2, :],
                            rhs_xh8(t0 + t, mv),
                            start=(t == 0), stop=(t == tc - 1),
                            perf_mode=DR)

            def mm16(ps2, wt, rhs, kc, koff=0):
                # k outer / bh inner: each stationary tile feeds 2 matmuls
                for k in range(kc):
                    for bh in range(2):
                        mv = slice(bh * NH, (bh + 1) * NH)
                        nc.tensor.matmul(
                            ps2[bh][:], wt[:, k * P:(k + 1) * P],
                            rhs(koff + k, mv),
                            start=(k == 0), stop=(k == kc - 1))

            # ---- phase A1: a1 (fp8), deferred tiny a2 matmuls ----
            ps_a2 = [psum_a2_pool.tile([1, NH], F32, tag="a20", name="psa20"),
                     psum_a2_pool.tile([1, NH], F32, tag="a21", name="psa21")]
            pend = []

            def flush_a2():
                jq, pair = pend.pop(0)
                for bh in range(2):
                    nc.tensor.matmul(ps_a2[bh][:], a2_sb[:, jq:jq + 1],
                                     pair[bh][:], start=(jq == 0),
                                     stop=(jq == JC - 1))

            for j in range(JC):
                wt = load_w8(3, j)
                ps2 = ps_pair("ps_a1_")
                mm8(ps2, wt)
                pair = []
                for bh in range(2):
                    a1b = a1_pool.tile([P, NH], BF16, tag="a1", name="a1b")
                    nc.scalar.activation(a1b[:], ps2[bh][:], AF.Relu,
                                         bias=bias_ap(5, j), scale=1.0 / WS)
                    pair.append(a1b)
                    if dbg and j == 0 and bh == 0:
                        nc.scalar.dma_start(out=dbga1[:, :], in_=a1b[:])
                pend.append((j, pair))
                # defer the tiny a2 matmuls one j so PE never waits on ScalarE
                if len(pend) == 2:
                    flush_a2()
            while pend:
                flush_a2()

            # alpha = sigmoid(a2 @ a1relu + a2_b): [1, BL]; broadcast via
            # DRAM roundtrip that hides under the r1/r2 phases
            for bh in range(2):
                asb = a1_pool.tile([1, NH], F32, tag="a1", name="alpha_sb")
                nc.scalar.activation(asb[:], ps_a2[bh][:], AF.Sigmoid,
                                     bias=bias_sb[0:1, 9 * JC: 9 * JC + 1])
                nc.sync.dma_start(
                    out=alpha_dram[0:1, bh * NH:(bh + 1) * NH], in_=asb[:])
            alpha_rep = consts.tile([P, BL], F32, name="alpha_rep")
            nc.gpsimd.dma_start(
                out=alpha_rep[:], in_=alpha_dram[0:1, :].broadcast_to([P, BL]))
            if dbg:
                nc.scalar.dma_start(out=dbgal[:, :], in_=alpha_rep[:])

            # ---- phase A2: r1 = relu(h @ r1_w.T + b) in fp8 (moving = the
            # resident fp8 h-half). r1 evicts straight to e4m3 pair-tiles
            # so r2 can also run fp8 DoubleRow; r2 evicts bf16 for r3. ----
            r1q = [r1_pool.tile([P, QK, BL], FP8, tag="r1a", name="r1a"),
                   r1_pool.tile([P, QK, BL], FP8, tag="r1b", name="r1b")]
            for j in range(JC):
                wt = w8_pool.tile([P, KC1, P], FP8, tag="w8r", bufs=3,
                                  name=f"w8r1_{j}")
                nc.sync.dma_start(out=wt[:], in_=w8r1[j])
                ps2 = ps_pair("ps_r1_")
                mm8(ps2, wt, tc=KC1 // 2, t0=TC2 // 2)
                qq, jj = divmod(j, QK)
                for bh in range(2):
                    nc.scalar.activation(
                        r1q[qq][:, jj, bh * NH:(bh + 1) * NH],
                        ps2[bh][:], AF.Relu, bias=bias_ap(6, j),
                        scale=1.0 / WS)
                if j == 0:
                    load_bulk_xh16(alpha_rep[0:1, 0:1])
                    load_bulk_cp(alpha_rep[0:1, 0:1])
            if dbg:
                nc.scalar.dma_start(out=dbgr1[:, :], in_=r1q[0][:, 0, :])

            def rhs_r2(k, mv):
                return r2[k][:, mv]

            # ---- phase A3: r2 = relu(r1 @ r2_w.T + b) in fp8 DoubleRow ----
            r2 = []
            for j in range(JC):
                wt = w8_pool.tile([P, KC1, P], FP8, tag="w8r", bufs=3,
                                  name=f"w8r2_{j}")
                nc.sync.dma_start(out=wt[:], in_=w8r2[j])
                t_ = r2_pool.tile([P, BL], BF16, tag=f"r2_{j}", name=f"r2_{j}")
                ps2 = ps_pair("ps_r2_")
                for t in range(KC1 // 2):
                    for bh in range(2):
                        mv = slice(bh * NH, (bh + 1) * NH)
                        nc.tensor.matmul(
                            ps2[bh][:], wt[:, 2 * t:2 * t + 2, :],
                            r1q[t // 2][:, (2 * t) % QK:(2 * t) % QK + 2, mv],
                            start=(t == 0), stop=(t == KC1 // 2 - 1),
                            perf_mode=DR)
                for bh in range(2):
                    nc.scalar.activation(t_[:, bh * NH:(bh + 1) * NH], ps2[bh][:],
                                         AF.Relu, bias=bias_ap(7, j),
                                         scale=1.0 / WS)
                r2.append(t_)
            if dbg:
                nc.scalar.dma_start(out=dbgr2[:, :], in_=r2[0][:])

            # ---- phase B: gates + r3 + combine, per feature tile j.
            # Order c,s,i (fp8), o, f (bf16), r3: the elementwise chain runs
            # while later matmuls stream; o comes before f/r3 so only the
            # short r3-evict -> add -> tanh -> mul chain trails the last MM.
            GATE8 = {"c": (0, 3, AF.Tanh), "s": (1, 4, AF.Sigmoid),
                     "i": (2, 0, AF.Sigmoid)}

            def gate8(key, j):
                gi, v, fn = GATE8[key]
                wt = load_w8(gi, j)
                t_ = g_pool.tile([P, BL], F32, tag=f"g8{key}", name=f"g8{key}")
                ps2 = ps_pair("ps_g8")
                mm8(ps2, wt)
                for bh in range(2):
                    nc.scalar.activation(t_[:, bh * NH:(bh + 1) * NH],
                                         ps2[bh][:], fn, bias=bias_ap(v, j),
                                         scale=1.0 / WS)
                return t_

            def gate16(gi, v, j):
                wt = load_wg16(gi, j)
                t_ = g_pool.tile([P, BL], F32, tag=f"g16{gi}", name=f"g16{gi}")
                ps2 = ps_pair("ps_g16")
                mm16(ps2, wt, rhs_xh16, KC2)
                for bh in range(2):
                    nc.scalar.activation(t_[:, bh * NH:(bh + 1) * NH],
                                         ps2[bh][:], AF.Sigmoid,
                                         bias=bias_ap(v, j))
                return t_

            for j in range(JC):
                ch = gate8("c", j)
                st = gate8("s", j)
                it = gate8("i", j)

                t1s = []
                for bh in range(2):
                    mv = slice(bh * NH, (bh + 1) * NH)
                    t1 = ew_pool.tile([P, NH], F32, tag=f"t1{bh}", name="t1")
                    nc.vector.tensor_mul(t1[:], it[:, mv], ch[:, mv])
                    nc.vector.tensor_mul(t1[:], t1[:], st[:, mv])
                    nc.vector.tensor_mul(t1[:], t1[:], alpha_rep[:, mv])
                    t1s.append(t1)

                ot = gate16(1, 2, j)

                ft = gate16(0, 1, j)
                for bh in range(2):
                    mv = slice(bh * NH, (bh + 1) * NH)
                    qq, kk = divmod(j, QK)
                    t2 = ew_pool.tile([P, NH], F32, tag=f"t2{bh}", name="t2",
                                      bufs=1)
                    nc.vector.tensor_mul(t2[:], ft[:, mv],
                                         cpq[qq][:, kk, mv])
                    nc.vector.tensor_add(t1s[bh][:], t1s[bh][:], t2[:])

                wt = load_wr16(2, j)
                ps2 = ps_pair("ps_r3_")
                mm16(ps2, wt, rhs_r2, KC1)
                if dbg and j == 0:
                    r3d = g_pool.tile([P, BL], F32, tag="r3d", name="r3d")
                    for bh in range(2):
                        nc.scalar.activation(
                            r3d[:, bh * NH:(bh + 1) * NH], ps2[bh][:],
                            AF.Identity, bias=bias_ap(8, j))
                    for gi, gt in enumerate([ch, st, it, ot, ft, r3d]):
                        nc.scalar.dma_start(out=dbgg[gi], in_=gt[:])
                for bh in range(2):
                    mv = slice(bh * NH, (bh + 1) * NH)
                    # stage[:, 0, :] = h, stage[:, 1, :] = c -> single store.
                    # c = (r3_psum + r3_bias) + t1 in one DVE op straight
                    # from PSUM: no ScalarE eviction on the r3 path.
                    stg = ew_pool.tile([P, 2, NH], BF16, tag=f"st{bh}",
                                       name="stg")
                    nc.vector.scalar_tensor_tensor(
                        stg[:, 1, :], ps2[bh][:], bias_ap(8, j), t1s[bh][:],
                        mybir.AluOpType.add, mybir.AluOpType.add)
                    th = ew_pool.tile([P, NH], F32, tag=f"th{bh}", name="th",
                                      bufs=1)
                    nc.scalar.activation(th[:], stg[:, 1, :], AF.Tanh)
                    nc.vector.tensor_mul(stg[:, 0, :], ot[:, mv], th[:])
                    nc.scalar.dma_start(out=out[j * P:(j + 1) * P, :, mv],
                                        in_=stg[:])

    nc.finalize()
    return nc


def _pack_w(W, kdim):
    # pack[j, p, k*128+m] = W[j*128+m, k*128+p]
    kc = kdim // P
    return np.ascontiguousarray(
        np.asarray(W, np.float32).reshape(JC, P, kc, P)
        .transpose(0, 3, 2, 1).reshape(JC, P, kc * P))


def _pack_act(aT, nq, qk=QK):
    # aT: [nq*qk*P, BL] -> [nq, P, qk, BL] with [q, p, kk, n] = aT[(q*qk+kk)*P+p, n]
    return np.ascontiguousarray(
        aT.reshape(nq, qk, P, BL).transpose(0, 2, 1, 3))


def _prepare(inputs):
    f = lambda name: np.asarray(inputs[name], dtype=np.float32)

    def comb(g):
        u = "U" + g[1]
        return np.concatenate([f(g + "_w"), f(u + "_w")], axis=1)

    # fp8 gates: c, s, i, a1 (order matches in-kernel GATE8/a1 indices)
    w8 = np.stack([
        _pack_w(comb("Wc") * WS, K2),
        _pack_w(comb("Ws") * WS, K2),
        _pack_w(comb("Wi") * WS, K2),
        _pack_w(f("a1_w") * WS, K2),
    ]).astype(E4NP).reshape(4, JC, P, KC2, P)
    w8r1 = (_pack_w(f("r1_w") * WS, D).astype(E4NP)
            .reshape(JC, P, KC1, P))
    w8r2 = (_pack_w(f("r2_w") * WS, D).astype(E4NP)
            .reshape(JC, P, KC1, P))
    # bf16 gates: f, o
    wf_ = np.stack([_pack_w(comb("Wf"), K2),
                    _pack_w(comb("Wo"), K2)]).astype(BFNP)
    wr_ = np.stack(
        [_pack_w(f(n + "_w"), D) for n in ("r1", "r2", "r3")]).astype(BFNP)
    a2p = np.ascontiguousarray(f("a2_w").reshape(KC1, P).T).astype(BFNP)

    bias_vecs = []
    for g in ("Wi", "Wf", "Wo", "Wc", "Ws"):
        u = "U" + g[1]
        bias_vecs.append(f(g + "_b") + f(u + "_b"))
    bias_vecs += [f("a1_b"), f("r1_b"), f("r2_b"), f("r3_b"),
                  np.full(D, f("a2_b")[0], np.float32)]
    # biasp[p, v*JC + j] = vec_v[j*128 + p]
    biasp = np.ascontiguousarray(
        np.stack(bias_vecs).reshape(10, JC, P).transpose(2, 0, 1)
        .reshape(P, 10 * JC))

    x, h, c = f("x"), f("h_prev"), f("c_prev")
    shared = {"w8": w8, "w8r1": w8r1, "w8r2": w8r2, "wf": wf_, "wr": wr_,
              "a2p": a2p, "biasp": biasp}
    in_maps = []
    for core in range(NCORES):
        sl = slice(core * BL, (core + 1) * BL)
        xhT = np.ascontiguousarray(
            np.concatenate([x[sl].T, h[sl].T], axis=0))  # [K2, BL]
        in_maps.append({**shared,
                        "xh16s": _pack_act(xhT.astype(BFNP), 4),
                        "xh8s": _pack_act(xhT.astype(E4NP), 4),
                        "cTs": _pack_act(
                            np.ascontiguousarray(c[sl].T).astype(BFNP), 2)})
    return in_maps


def _run(inputs, trace=False):
    from concourse.bass_utils import run_bass_kernel_spmd

    if "nc" not in _CACHE:
        _CACHE["nc"] = _build()
    nc = _CACHE["nc"]
    in_maps = _prepare(inputs)
    res = run_bass_kernel_spmd(nc, in_maps, core_ids=list(range(NCORES)),
                               trace=trace)
    h = np.empty((B, D), np.float32)
    c = np.empty((B, D), np.float32)
    for core in range(NCORES):
        o = res.results[core]["out"]  # [D, 2, BL] bf16
        sl = slice(core * BL, (core + 1) * BL)
        h[sl] = o[:, 0].T.astype(np.float32)
        c[sl] = o[:, 1].T.astype(np.float32)
    return (h, c), res


def kernel(**inputs):
    (h, c), _ = _run(inputs, trace=False)
    return (h, c)


# revision 76
# speedup vs baseline: 1.2153x; 1.0183x over previous
"""AdaptiveLSTMCellWithRes on 8 TRN2 NeuronCores.

Data-parallel over batch (1024 rows/core), weights replicated.
All on-chip compute happens in transposed-activation space [feat, batch].
Mixed precision:
  - i, s, c_hat, a1 matmuls run fp8(e4m3) with DoubleRow perf mode
    (2 k-tiles per PE pass, 2x the bf16 rate). Weights are pre-scaled
    by 64 on host (0.02-std values would land subnormal in e4m3);
    the 1/64 folds into the PSUM-evicting activation's scale.
  - f, o, residual chain and a2 (the error-critical terms) run bf16.
  - PSUM, biases and the elementwise combine stay fp32; h/c outputs
    are written bf16 (well inside the error budget, halves store DMA).
DMA dispatch costs ~0.6us per dma_start on the issuing engine, so
activations/c_prev/outputs ride the Scalar HWDGE queue in a few big
host-packed transfers while weight slabs stream on the Sync queue.
"""

import sys

if "/opt/trn_rl_repo" not in sys.path:
    sys.path.insert(0, "/opt/trn_rl_repo")

import numpy as np
import ml_dtypes

P = 128
B = 8192          # global batch
NCORES = 8
BL = B // NCORES  # batch per core (1024)
D = 1024          # feature dim
K2 = 2048         # concat(x, h) contraction
JC = D // P       # 8 output-feature tiles
KC2 = K2 // P     # 16 k-chunks for gates/a1
KC1 = D // P      # 8 k-chunks for residual/a2
TC2 = KC2 // 2    # 8 fp8 double-row pair steps
NH = BL // 2      # moving free dim per matmul (512)
WS = 64.0         # fp8 weight pre-scale
QK = 4            # k-chunks per packed activation quarter-tile

E4NP = ml_dtypes.float8_e4m3
BFNP = ml_dtypes.bfloat16

_CACHE = {}


def _build(dbg=False):
    import concourse.bass as bass  # noqa: F401
    from concourse import bacc, mybir
    import concourse.tile as tile

    F32 = mybir.dt.float32
    BF16 = mybir.dt.bfloat16
    FP8 = mybir.dt.float8e4
    AF = mybir.ActivationFunctionType
    DR = mybir.MatmulPerfMode.DoubleRow

    nc = bacc.Bacc()

    # fp8 gates (c, s, i, a1): pack[g, j, p, kk, m] = e4m3(WS * Wg[j*128+m, kk*128+p])
    w8 = nc.declare_dram_parameter("w8", [4, JC, P, KC2, P], FP8, isOutput=False)
    # fp8 r1/r2 weights, same packing with K=D
    w8r1 = nc.declare_dram_parameter("w8r1", [JC, P, KC1, P], FP8,
                                     isOutput=False)
    w8r2 = nc.declare_dram_parameter("w8r2", [JC, P, KC1, P], FP8,
                                     isOutput=False)
    # bf16 gates (f, o): pack[g, j, p, k*128+m] = W[j*128+m, k*128+p]
    wf = nc.declare_dram_parameter("wf", [2, JC, P, K2], BF16, isOutput=False)
    # residual weights (r1, r2, r3): packed [3, JC, P, D] bf16
    wr = nc.declare_dram_parameter("wr", [3, JC, P, D], BF16, isOutput=False)
    # a2 weight: [P, KC1] bf16 with a2p[p, k] = a2_w[0, k*128+p]
    a2p = nc.declare_dram_parameter("a2p", [P, KC1], BF16, isOutput=False)
    # biases: [P, 10*JC]; col v*JC+j holds vec_v[j*128:(j+1)*128]
    # v: 0..4 = combined gate biases (i,f,o,c,s), 5=a1_b, 6=r1_b, 7=r2_b,
    # 8=r3_b, 9=a2_b (replicated)
    biasp = nc.declare_dram_parameter("biasp", [P, 10 * JC], F32, isOutput=False)
    # activations pre-swizzled into quarter tiles: [q, p, kk, n] =
    # act[(q*QK+kk)*128+p, n]; q 0..1 = x^T, 2..3 = h^T
    xh16s = nc.declare_dram_parameter("xh16s", [4, P, QK, BL], BF16,
                                      isOutput=False)
    xh8s = nc.declare_dram_parameter("xh8s", [4, P, QK, BL], FP8,
                                     isOutput=False)
    cTs = nc.declare_dram_parameter("cTs", [2, P, QK, BL], BF16, isOutput=False)
    # out[d, 0, n] = h_t^T, out[d, 1, n] = c_t^T (bf16)
    out = nc.declare_dram_parameter("out", [D, 2, BL], BF16, isOutput=True)

    alpha_dram = nc.dram_tensor("alpha_dram", [1, BL], F32)

    if dbg:
        dbg8 = nc.declare_dram_parameter("dbg8", [P, 2, BL], FP8, isOutput=True)
        dbga1 = nc.declare_dram_parameter("dbga1", [P, NH], BF16, isOutput=True)
        dbgal = nc.declare_dram_parameter("dbgal", [P, BL], F32, isOutput=True)
        dbgr1 = nc.declare_dram_parameter("dbgr1", [P, BL], FP8, isOutput=True)
        dbgr2 = nc.declare_dram_parameter("dbgr2", [P, BL], BF16, isOutput=True)
        dbgg = nc.declare_dram_parameter("dbgg", [6, P, BL], F32, isOutput=True)

    with tile.TileContext(nc) as tc:
        with (
            tc.tile_pool(name="consts", bufs=1) as consts,
            tc.tile_pool(name="xh8", bufs=1) as xh8_pool,
            tc.tile_pool(name="xh16", bufs=1) as xh16_pool,
            tc.tile_pool(name="cpre", bufs=1) as cp_pool,
            tc.tile_pool(name="w8p", bufs=6) as w8_pool,
            tc.tile_pool(name="wfp", bufs=4) as wf_pool,
            tc.tile_pool(name="a1s", bufs=4) as a1_pool,
            tc.tile_pool(name="r1", bufs=1) as r1_pool,
            tc.tile_pool(name="r2", bufs=1) as r2_pool,
            tc.tile_pool(name="gates", bufs=1) as g_pool,
            tc.tile_pool(name="ew", bufs=2) as ew_pool,
            tc.tile_pool(name="psum", bufs=3, space="PSUM") as psum_pool,
            tc.tile_pool(name="psum_a2", bufs=1, space="PSUM") as psum_a2_pool,
        ):
            bias_sb = consts.tile([P, 10 * JC], F32, name="bias_sb")
            a2_sb = consts.tile([P, KC1], BF16, name="a2_sb")

            def bias_ap(v, j):
                return bias_sb[:, v * JC + j: v * JC + j + 1]

            def load_w8(g, j):
                wt = w8_pool.tile([P, KC2, P], FP8, tag="w8", name=f"w8_{g}_{j}")
                nc.sync.dma_start(out=wt[:], in_=w8[g, j])
                return wt

            # ---- prefix. Scalar queue: ONLY the fp8 xh stream (a1/r1 are
            # the first phases) — DMA dispatches share the ScalarE
            # instruction stream with PSUM evictions, so it must stay
            # near-empty. Sync: consts then in-loop weight slabs. GpSimd
            # (idle otherwise): all bulk loads needed only in phase B.
            nc.sync.dma_start(out=bias_sb[:], in_=biasp[:, :])
            nc.sync.dma_start(out=a2_sb[:], in_=a2p[:, :])
            xh8q = []
            for q in range(4):
                tl = xh8_pool.tile([P, QK, BL], FP8, tag=f"x8{q}", name=f"x8{q}")
                nc.scalar.dma_start(out=tl[:], in_=xh8s[q])
                xh8q.append(tl)
            xh16q = [None] * 4
            cpq = []

            def load_bulk_xh16(dep_ap):
                # phase-B bulk loads on the otherwise-idle gpsimd queue.
                # The scheduler orders DMAs by dependency, not program
                # order, so a tiny write sourced from an early phase-A
                # product (overwritten by the DMA) holds these transfers
                # back until the critical fp8 prefix stream has landed.
                for q in (0, 1, 2, 3):
                    tl = xh16_pool.tile([P, QK, BL], BF16, tag=f"x{q}",
                                        name=f"x{q}")
                    nc.scalar.copy(tl[0:1, 0, 0:1], dep_ap)
                    nc.gpsimd.dma_start(out=tl[:], in_=xh16s[q])
                    xh16q[q] = tl

            def load_bulk_cp(dep_ap):
                for q in range(2):
                    tl = cp_pool.tile([P, QK, BL], BF16, tag=f"cp{q}",
                                      name=f"cp{q}")
                    nc.scalar.copy(tl[0:1, 0, 0:1], dep_ap)
                    nc.gpsimd.dma_start(out=tl[:], in_=cTs[q])
                    cpq.append(tl)

            if dbg:
                nc.scalar.dma_start(out=dbg8[:, :, :], in_=xh8q[0][:, 0:2, :])

            def rhs_xh8(t, mv):
                q, kk = divmod(2 * t, QK)
                return xh8q[q][:, kk:kk + 2, mv]

            def rhs_xh16(k, mv):
                q, kk = divmod(k, QK)
                return xh16q[q][:, kk:kk + 1, mv]

            def load_wg16(g, j):
                wt = wf_pool.tile([P, K2], BF16, tag="wg", name=f"wg_{g}_{j}",
                                  bufs=3)
                nc.sync.dma_start(out=wt[:], in_=wf[g, j])
                return wt

            def load_wr16(ri, j):
                wt = wf_pool.tile([P, D], BF16, tag="wr", name=f"wr_{ri}_{j}",
                                  bufs=6)
                nc.sync.dma_start(out=wt[:], in_=wr[ri, j])
                return wt

            def ps_pair(nm):
                return [psum_pool.tile([P, NH], F32, tag="ps0", name=f"{nm}0"),
                        psum_pool.tile([P, NH], F32, tag="ps1", name=f"{nm}1")]

            def mm8(ps2, wt, tc=TC2, t0=0):
                # fp8 DoubleRow over tc k-pairs starting at pair t0; each
                # stationary pair feeds both batch halves
                for t in range(tc):
                    for bh in range(2):
                        mv = slice(bh * NH, (bh + 1) * NH)
                        nc.tensor.matmul(
                            ps2[bh][:], wt[:, 2 * t:2 * t + 2, :],
                            rhs_xh8(t0 + t, mv),
                            start=(t == 0), stop=(t == tc - 1),
                            perf_mode=DR)

            def mm16(ps2, wt, rhs, kc, koff=0):
                # k outer / bh inner: each stationary tile feeds 2 matmuls
                for k in range(kc):
                    for bh in range(2):
                        mv = slice(bh * NH, (bh + 1) * NH)
                        nc.tensor.matmul(
                            ps2[bh][:], wt[:, k * P:(k + 1) * P],
                            rhs(koff + k, mv),
                            start=(k == 0), stop=(k == kc - 1))

            # ---- phase A1: a1 (fp8), deferred tiny a2 matmuls ----
            ps_a2 = [psum_a2_pool.tile([1, NH], F32, tag="a20", name="psa20"),
                     psum_a2_pool.tile([1, NH], F32, tag="a21", name="psa21")]
            pend = []

            def flush_a2():
                jq, pair = pend.pop(0)
                for bh in range(2):
                    nc.tensor.matmul(ps_a2[bh][:], a2_sb[:, jq:jq + 1],
                                     pair[bh][:], start=(jq == 0),
                                     stop=(jq == JC - 1))

            for j in range(JC):
                wt = load_w8(3, j)
                ps2 = ps_pair("ps_a1_")
                mm8(ps2, wt)
                pair = []
                for bh in range(2):
                    a1b = a1_pool.tile([P, NH], BF16, tag="a1", name="a1b")
                    nc.scalar.activation(a1b[:], ps2[bh][:], AF.Relu,
                                         bias=bias_ap(5, j), scale=1.0 / WS)
                    pair.append(a1b)
                    if dbg and j == 0 and bh == 0:
                        nc.scalar.dma_start(out=dbga1[:, :], in_=a1b[:])
                pend.append((j, pair))
                # defer the tiny a2 matmuls one j so PE never waits on ScalarE
                if len(pend) == 2:
                    flush_a2()
            while pend:
                flush_a2()

            # alpha = sigmoid(a2 @ a1relu + a2_b): [1, BL]; broadcast via
            # DRAM roundtrip that hides under the r1/r2 phases
            for bh in range(2):
                asb = a1_pool.tile([1, NH], F32, tag="a1", name="alpha_sb")
                nc.scalar.activation(asb[:], ps_a2[bh][:], AF.Sigmoid,
                                     bias=bias_sb[0:1, 9 * JC: 9 * JC + 1])
                nc.sync.dma_start(
                    out=alpha_dram[0:1, bh * NH:(bh + 1) * NH], in_=asb[:])
            alpha_rep = consts.tile([P, BL], F32, name="alpha_rep")
            nc.gpsimd.dma_start(
                out=alpha_rep[:], in_=alpha_dram[0:1, :].broadcast_to([P, BL]))
            if dbg:
                nc.scalar.dma_start(out=dbgal[:, :], in_=alpha_rep[:])

            # ---- phase A2: r1 = relu(h @ r1_w.T + b) in fp8 (moving = the
            # resident fp8 h-half). r1 evicts straight to e4m3 pair-tiles
            # so r2 can also run fp8 DoubleRow; r2 evicts bf16 for r3. ----
            r1q = [r1_pool.tile([P, QK, BL], FP8, tag="r1a", name="r1a"),
                   r1_pool.tile([P, QK, BL], FP8, tag="r1b", name="r1b")]
            for j in range(JC):
                wt = w8_pool.tile([P, KC1, P], FP8, tag="w8r", bufs=3,
                                  name=f"w8r1_{j}")
                nc.sync.dma_start(out=wt[:], in_=w8r1[j])
                ps2 = ps_pair("ps_r1_")
                mm8(ps2, wt, tc=KC1 // 2, t0=TC2 // 2)
                qq, jj = divmod(j, QK)
                for bh in range(2):
                    nc.scalar.activation(
                        r1q[qq][:, jj, bh * NH:(bh + 1) * NH],
                        ps2[bh][:], AF.Relu, bias=bias_ap(6, j),
                        scale=1.0 / WS)
                if j == 0:
                    load_bulk_xh16(alpha_rep[0:1, 0:1])
                    load_bulk_cp(alpha_rep[0:1, 0:1])
            if dbg:
                nc.scalar.dma_start(out=dbgr1[:, :], in_=r1q[0][:, 0, :])

            def rhs_r2(k, mv):
                return r2[k][:, mv]

            # ---- phase A3: r2 = relu(r1 @ r2_w.T + b) in fp8 DoubleRow ----
            r2 = []
            for j in range(JC):
                wt = w8_pool.tile([P, KC1, P], FP8, tag="w8r2", bufs=4,
                                  name=f"w8r2_{j}")
                nc.sync.dma_start(out=wt[:], in_=w8r2[j])
                t_ = r2_pool.tile([P, BL], BF16, tag=f"r2_{j}", name=f"r2_{j}")
                ps2 = ps_pair("ps_r2_")
                for t in range(KC1 // 2):
                    for bh in range(2):
                        mv = slice(bh * NH, (bh + 1) * NH)
                        nc.tensor.matmul(
                            ps2[bh][:], wt[:, 2 * t:2 * t + 2, :],
                            r1q[t // 2][:, (2 * t) % QK:(2 * t) % QK + 2, mv],
                            start=(t == 0), stop=(t == KC1 // 2 - 1),
                            perf_mode=DR)
                for bh in range(2):
                    nc.scalar.activation(t_[:, bh * NH:(bh + 1) * NH], ps2[bh][:],
                                         AF.Relu, bias=bias_ap(7, j),
                                         scale=1.0 / WS)
                r2.append(t_)
            if dbg:
                nc.scalar.dma_start(out=dbgr2[:, :], in_=r2[0][:])

            # ---- phase B: gates + r3 + combine, per feature tile j.
            # Order c,s,i (fp8), o, f (bf16), r3: the elementwise chain runs
            # while later matmuls stream; o comes before f/r3 so only the
            # short r3-evict -> add -> tanh -> mul chain trails the last MM.
            GATE8 = {"c": (0, 3, AF.Tanh), "s": (1, 4, AF.Sigmoid),
                     "i": (2, 0, AF.Sigmoid)}

            def gate8(key, j):
                gi, v, fn = GATE8[key]
                wt = load_w8(gi, j)
                t_ = g_pool.tile([P, BL], F32, tag=f"g8{key}", name=f"g8{key}")
                ps2 = ps_pair("ps_g8")
                mm8(ps2, wt)
                for bh in range(2):
                    nc.scalar.activation(t_[:, bh * NH:(bh + 1) * NH],
                                         ps2[bh][:], fn, bias=bias_ap(v, j),
                                         scale=1.0 / WS)
                return t_

            def gate16(gi, v, j):
                wt = load_wg16(gi, j)
                t_ = g_pool.tile([P, BL], F32, tag=f"g16{gi}", name=f"g16{gi}")
                ps2 = ps_pair("ps_g16")
                mm16(ps2, wt, rhs_xh16, KC2)
                for bh in range(2):
                    nc.scalar.activation(t_[:, bh * NH:(bh + 1) * NH],
                                         ps2[bh][:], AF.Sigmoid,
                                         bias=bias_ap(v, j))
                return t_

            for j in range(JC):
                ch = gate8("c", j)
                st = gate8("s", j)
                it = gate8("i", j)

                t1s = []
                for bh in range(2):
                    mv = slice(bh * NH, (bh + 1) * NH)
                    t1 = ew_pool.tile([P, NH], F32, tag=f"t1{bh}", name="t1")
                    nc.vector.tensor_mul(t1[:], it[:, mv], ch[:, mv])
                    nc.vector.tensor_mul(t1[:], t1[:], st[:, mv])
                    nc.vector.tensor_mul(t1[:], t1[:], alpha_rep[:, mv])
                    t1s.append(t1)

                ot = gate16(1, 2, j)

                ft = gate16(0, 1, j)
                for bh in range(2):
                    mv = slice(bh * NH, (bh + 1) * NH)
                    qq, kk = divmod(j, QK)
                    t2 = ew_pool.tile([P, NH], F32, tag=f"t2{bh}", name="t2",
                                      bufs=1)
                    nc.vector.tensor_mul(t2[:], ft[:, mv],
                                         cpq[qq][:, kk, mv])
                    nc.vector.tensor_add(t1s[bh][:], t1s[bh][:], t2[:])

                wt = load_wr16(2, j)
                ps2 = ps_pair("ps_r3_")
                mm16(ps2, wt, rhs_r2, KC1)
                if dbg and j == 0:
                    r3d = g_pool.tile([P, BL], F32, tag="r3d", name="r3d")
                    for bh in range(2):
                        nc.scalar.activation(
                            r3d[:, bh * NH:(bh + 1) * NH], ps2[bh][:],
                            AF.Identity, bias=bias_ap(8, j))
                    for gi, gt in enumerate([ch, st, it, ot, ft, r3d]):
                        nc.scalar.dma_start(out=dbgg[gi], in_=gt[:])
                for bh in range(2):
                    mv = slice(bh * NH, (bh + 1) * NH)
                    # stage[:, 0, :] = h, stage[:, 1, :] = c -> single store.
                    # c = (r3_psum + r3_bias) + t1 in one DVE op straight
                    # from PSUM: no ScalarE eviction on the r3 path.
                    stg = ew_pool.tile([P, 2, NH], BF16, tag=f"st{bh}",
                                       name="stg")
                    nc.vector.scalar_tensor_tensor(
                        stg[:, 1, :], ps2[bh][:], bias_ap(8, j), t1s[bh][:],
                        mybir.AluOpType.add, mybir.AluOpType.add)
                    th = ew_pool.tile([P, NH], F32, tag=f"th{bh}", name="th",
                                      bufs=1)
                    nc.scalar.activation(th[:], stg[:, 1, :], AF.Tanh)
                    nc.vector.tensor_mul(stg[:, 0, :], ot[:, mv], th[:])
                    nc.scalar.dma_start(out=out[j * P:(j + 1) * P, :, mv],
                                        in_=stg[:])

    nc.finalize()
    return nc


def _pack_w(W, kdim):
    # pack[j, p, k*128+m] = W[j*128+m, k*128+p]
    kc = kdim // P
    return np.ascontiguousarray(
        np.asarray(W, np.float32).reshape(JC, P, kc, P)
        .transpose(0, 3, 2, 1).reshape(JC, P, kc * P))


def _pack_act(aT, nq, qk=QK):
    # aT: [nq*qk*P, BL] -> [nq, P, qk, BL] with [q, p, kk, n] = aT[(q*qk+kk)*P+p, n]
    return np.ascontiguousarray(
        aT.reshape(nq, qk, P, BL).transpose(0, 2, 1, 3))


def _prepare(inputs):
    f = lambda name: np.asarray(inputs[name], dtype=np.float32)

    def comb(g):
        u = "U" + g[1]
        return np.concatenate([f(g + "_w"), f(u + "_w")], axis=1)

    # fp8 gates: c, s, i, a1 (order matches in-kernel GATE8/a1 indices)
    w8 = np.stack([
        _pack_w(comb("Wc") * WS, K2),
        _pack_w(comb("Ws") * WS, K2),
        _pack_w(comb("Wi") * WS, K2),
        _pack_w(f("a1_w") * WS, K2),
    ]).astype(E4NP).reshape(4, JC, P, KC2, P)
    w8r1 = (_pack_w(f("r1_w") * WS, D).astype(E4NP)
            .reshape(JC, P, KC1, P))
    w8r2 = (_pack_w(f("r2_w") * WS, D).astype(E4NP)
            .reshape(JC, P, KC1, P))
    # bf16 gates: f, o
    wf_ = np.stack([_pack_w(comb("Wf"), K2),
                    _pack_w(comb("Wo"), K2)]).astype(BFNP)
    wr_ = np.stack(
        [_pack_w(f(n + "_w"), D) for n in ("r1", "r2", "r3")]).astype(BFNP)
    a2p = np.ascontiguousarray(f("a2_w").reshape(KC1, P).T).astype(BFNP)

    bias_vecs = []
    for g in ("Wi", "Wf", "Wo", "Wc", "Ws"):
        u = "U" + g[1]
        bias_vecs.append(f(g + "_b") + f(u + "_b"))
    bias_vecs += [f("a1_b"), f("r1_b"), f("r2_b"), f("r3_b"),
                  np.full(D, f("a2_b")[0], np.float32)]
    # biasp[p, v*JC + j] = vec_v[j*128 + p]
    biasp = np.ascontiguousarray(
        np.stack(bias_vecs).reshape(10, JC, P).transpose(2, 0, 1)
        .reshape(P, 10 * JC))

    x, h, c = f("x"), f("h_prev"), f("c_prev")
    shared = {"w8": w8, "w8r1": w8r1, "w8r2": w8r2, "wf": wf_, "wr": wr_,
              "a2p": a2p, "biasp": biasp}
    in_maps = []
    for core in range(NCORES):
        sl = slice(core * BL, (core + 1) * BL)
        xhT = np.ascontiguousarray(
            np.concatenate([x[sl].T, h[sl].T], axis=0))  # [K2, BL]
        in_maps.append({**shared,
                        "xh16s": _pack_act(xhT.astype(BFNP), 4),
                        "xh8s": _pack_act(xhT.astype(E4NP), 4),
                        "cTs": _pack_act(
                            np.ascontiguousarray(c[sl].T).astype(BFNP), 2)})
    return in_maps


def _run(inputs, trace=False):
    from concourse.bass_utils import run_bass_kernel_spmd

    if "nc" not in _CACHE:
        _CACHE["nc"] = _build()
    nc = _CACHE["nc"]
    in_maps = _prepare(inputs)
    res = run_bass_kernel_spmd(nc, in_maps, core_ids=list(range(NCORES)),
                               trace=trace)
    h = np.empty((B, D), np.float32)
    c = np.empty((B, D), np.float32)
    for core in range(NCORES):
        o = res.results[core]["out"]  # [D, 2, BL] bf16
        sl = slice(core * BL, (core + 1) * BL)
        h[sl] = o[:, 0].T.astype(np.float32)
        c[sl] = o[:, 1].T.astype(np.float32)
    return (h, c), res


def kernel(**inputs):
    (h, c), _ = _run(inputs, trace=False)
    return (h, c)


# revision 77
# speedup vs baseline: 1.2583x; 1.0354x over previous
"""AdaptiveLSTMCellWithRes on 8 TRN2 NeuronCores.

Data-parallel over batch (1024 rows/core), weights replicated.
All on-chip compute happens in transposed-activation space [feat, batch].
Mixed precision:
  - i, s, c_hat, a1 matmuls run fp8(e4m3) with DoubleRow perf mode
    (2 k-tiles per PE pass, 2x the bf16 rate). Weights are pre-scaled
    by 64 on host (0.02-std values would land subnormal in e4m3);
    the 1/64 folds into the PSUM-evicting activation's scale.
  - f, o, residual chain and a2 (the error-critical terms) run bf16.
  - PSUM, biases and the elementwise combine stay fp32; h/c outputs
    are written bf16 (well inside the error budget, halves store DMA).
DMA dispatch costs ~0.6us per dma_start on the issuing engine, so
activations/c_prev/outputs ride the Scalar HWDGE queue in a few big
host-packed transfers while weight slabs stream on the Sync queue.
"""

import sys

if "/opt/trn_rl_repo" not in sys.path:
    sys.path.insert(0, "/opt/trn_rl_repo")

import numpy as np
import ml_dtypes

P = 128
B = 8192          # global batch
NCORES = 8
BL = B // NCORES  # batch per core (1024)
D = 1024          # feature dim
K2 = 2048         # concat(x, h) contraction
JC = D // P       # 8 output-feature tiles
KC2 = K2 // P     # 16 k-chunks for gates/a1
KC1 = D // P      # 8 k-chunks for residual/a2
TC2 = KC2 // 2    # 8 fp8 double-row pair steps
NH = BL // 2      # moving free dim per matmul (512)
WS = 64.0         # fp8 weight pre-scale
QK = 4            # k-chunks per packed activation quarter-tile

E4NP = ml_dtypes.float8_e4m3
BFNP = ml_dtypes.bfloat16

_CACHE = {}


def _build(dbg=False):
    import concourse.bass as bass  # noqa: F401
    from concourse import bacc, mybir
    import concourse.tile as tile

    F32 = mybir.dt.float32
    BF16 = mybir.dt.bfloat16
    FP8 = mybir.dt.float8e4
    AF = mybir.ActivationFunctionType
    DR = mybir.MatmulPerfMode.DoubleRow

    nc = bacc.Bacc()

    # fp8 gates (c, s, i, a1): pack[g, j, p, kk, m] = e4m3(WS * Wg[j*128+m, kk*128+p])
    w8 = nc.declare_dram_parameter("w8", [4, JC, P, KC2, P], FP8, isOutput=False)
    # fp8 r1/r2 weights, same packing with K=D
    w8r1 = nc.declare_dram_parameter("w8r1", [JC, P, KC1, P], FP8,
                                     isOutput=False)
    w8r2 = nc.declare_dram_parameter("w8r2", [JC, P, KC1, P], FP8,
                                     isOutput=False)
    # bf16 gates (f, o): pack[g, j, p, k*128+m] = W[j*128+m, k*128+p]
    wf = nc.declare_dram_parameter("wf", [2, JC, P, K2], BF16, isOutput=False)
    # residual weights (r1, r2, r3): packed [3, JC, P, D] bf16
    wr = nc.declare_dram_parameter("wr", [3, JC, P, D], BF16, isOutput=False)
    # a2 weight: [P, KC1] bf16 with a2p[p, k] = a2_w[0, k*128+p]
    a2p = nc.declare_dram_parameter("a2p", [P, KC1], BF16, isOutput=False)
    # biases: [P, 10*JC]; col v*JC+j holds vec_v[j*128:(j+1)*128]
    # v: 0..4 = combined gate biases (i,f,o,c,s), 5=a1_b, 6=r1_b, 7=r2_b,
    # 8=r3_b, 9=a2_b (replicated)
    biasp = nc.declare_dram_parameter("biasp", [P, 10 * JC], F32, isOutput=False)
    # activations pre-swizzled into quarter tiles: [q, p, kk, n] =
    # act[(q*QK+kk)*128+p, n]; q 0..1 = x^T, 2..3 = h^T
    xh16s = nc.declare_dram_parameter("xh16s", [4, P, QK, BL], BF16,
                                      isOutput=False)
    xh8s = nc.declare_dram_parameter("xh8s", [4, P, QK, BL], FP8,
                                     isOutput=False)
    cTs = nc.declare_dram_parameter("cTs", [2, P, QK, BL], BF16, isOutput=False)
    # out[d, 0, n] = h_t^T, out[d, 1, n] = c_t^T (bf16)
    out = nc.declare_dram_parameter("out", [D, 2, BL], BF16, isOutput=True)

    alpha_dram = nc.dram_tensor("alpha_dram", [1, BL], F32)

    if dbg:
        dbg8 = nc.declare_dram_parameter("dbg8", [P, 2, BL], FP8, isOutput=True)
        dbga1 = nc.declare_dram_parameter("dbga1", [P, NH], BF16, isOutput=True)
        dbgal = nc.declare_dram_parameter("dbgal", [P, BL], F32, isOutput=True)
        dbgr1 = nc.declare_dram_parameter("dbgr1", [P, BL], FP8, isOutput=True)
        dbgr2 = nc.declare_dram_parameter("dbgr2", [P, BL], BF16, isOutput=True)
        dbgg = nc.declare_dram_parameter("dbgg", [6, P, BL], F32, isOutput=True)

    with tile.TileContext(nc) as tc:
        with (
            tc.tile_pool(name="consts", bufs=1) as consts,
            tc.tile_pool(name="xh8", bufs=1) as xh8_pool,
            tc.tile_pool(name="xh16", bufs=1) as xh16_pool,
            tc.tile_pool(name="cpre", bufs=1) as cp_pool,
            tc.tile_pool(name="w8p", bufs=6) as w8_pool,
            tc.tile_pool(name="wfp", bufs=4) as wf_pool,
            tc.tile_pool(name="a1s", bufs=4) as a1_pool,
            tc.tile_pool(name="r1", bufs=1) as r1_pool,
            tc.tile_pool(name="r2", bufs=1) as r2_pool,
            tc.tile_pool(name="gates", bufs=1) as g_pool,
            tc.tile_pool(name="ew", bufs=2) as ew_pool,
            tc.tile_pool(name="psum", bufs=3, space="PSUM") as psum_pool,
            tc.tile_pool(name="psum_a2", bufs=1, space="PSUM") as psum_a2_pool,
        ):
            bias_sb = consts.tile([P, 10 * JC], F32, name="bias_sb")
            a2_sb = consts.tile([P, KC1], BF16, name="a2_sb")

            def bias_ap(v, j):
                return bias_sb[:, v * JC + j: v * JC + j + 1]

            def load_w8(g, j):
                wt = w8_pool.tile([P, KC2, P], FP8, tag="w8", name=f"w8_{g}_{j}")
                nc.sync.dma_start(out=wt[:], in_=w8[g, j])
                return wt

            # ---- prefix. Scalar queue: ONLY the fp8 xh stream (a1/r1 are
            # the first phases) — DMA dispatches share the ScalarE
            # instruction stream with PSUM evictions, so it must stay
            # near-empty. Sync: consts then in-loop weight slabs. GpSimd
            # (idle otherwise): all bulk loads needed only in phase B.
            nc.sync.dma_start(out=bias_sb[:], in_=biasp[:, :])
            nc.sync.dma_start(out=a2_sb[:], in_=a2p[:, :])
            xh8q = []
            for q in range(4):
                tl = xh8_pool.tile([P, QK, BL], FP8, tag=f"x8{q}", name=f"x8{q}")
                nc.scalar.dma_start(out=tl[:], in_=xh8s[q])
                xh8q.append(tl)
            xh16q = [None] * 4
            cpq = []

            def load_bulk_xh16(dep_ap):
                # phase-B bulk loads on the otherwise-idle gpsimd queue.
                # The scheduler orders DMAs by dependency, not program
                # order, so a tiny write sourced from an early phase-A
                # product (overwritten by the DMA) holds these transfers
                # back until the critical fp8 prefix stream has landed.
                for q in (0, 1, 2, 3):
                    tl = xh16_pool.tile([P, QK, BL], BF16, tag=f"x{q}",
                                        name=f"x{q}")
                    nc.scalar.copy(tl[0:1, 0, 0:1], dep_ap)
                    nc.gpsimd.dma_start(out=tl[:], in_=xh16s[q])
                    xh16q[q] = tl

            def load_bulk_cp(dep_ap):
                for q in range(2):
                    tl = cp_pool.tile([P, QK, BL], BF16, tag=f"cp{q}",
                                      name=f"cp{q}")
                    nc.scalar.copy(tl[0:1, 0, 0:1], dep_ap)
                    nc.gpsimd.dma_start(out=tl[:], in_=cTs[q])
                    cpq.append(tl)

            if dbg:
                nc.scalar.dma_start(out=dbg8[:, :, :], in_=xh8q[0][:, 0:2, :])

            def rhs_xh8(t, mv):
                q, kk = divmod(2 * t, QK)
                return xh8q[q][:, kk:kk + 2, mv]

            def rhs_xh16(k, mv):
                q, kk = divmod(k, QK)
                return xh16q[q][:, kk:kk + 1, mv]

            def load_wg16(g, j):
                wt = wf_pool.tile([P, K2], BF16, tag="wg", name=f"wg_{g}_{j}",
                                  bufs=3)
                nc.sync.dma_start(out=wt[:], in_=wf[g, j])
                return wt

            def load_wr16(ri, j):
                wt = wf_pool.tile([P, D], BF16, tag="wr", name=f"wr_{ri}_{j}",
                                  bufs=6)
                nc.sync.dma_start(out=wt[:], in_=wr[ri, j])
                return wt

            def ps_pair(nm):
                return [psum_pool.tile([P, NH], F32, tag="ps0", name=f"{nm}0"),
                        psum_pool.tile([P, NH], F32, tag="ps1", name=f"{nm}1")]

            def mm8(ps2, wt, tc=TC2, t0=0):
                # fp8 DoubleRow over tc k-pairs starting at pair t0; each
                # stationary pair feeds both batch halves
                for t in range(tc):
                    for bh in range(2):
                        mv = slice(bh * NH, (bh + 1) * NH)
                        nc.tensor.matmul(
                            ps2[bh][:], wt[:, 2 * t:2 * t + 2, :],
                            rhs_xh8(t0 + t, mv),
                            start=(t == 0), stop=(t == tc - 1),
                            perf_mode=DR)

            def mm16(ps2, wt, rhs, kc, koff=0):
                # k outer / bh inner: each stationary tile feeds 2 matmuls
                for k in range(kc):
                    for bh in range(2):
                        mv = slice(bh * NH, (bh + 1) * NH)
                        nc.tensor.matmul(
                            ps2[bh][:], wt[:, k * P:(k + 1) * P],
                            rhs(koff + k, mv),
                            start=(k == 0), stop=(k == kc - 1))

            # ---- phase A1: a1 (fp8), deferred tiny a2 matmuls ----
            ps_a2 = [psum_a2_pool.tile([1, NH], F32, tag="a20", name="psa20"),
                     psum_a2_pool.tile([1, NH], F32, tag="a21", name="psa21")]
            pend = []

            def flush_a2():
                jq, pair = pend.pop(0)
                for bh in range(2):
                    nc.tensor.matmul(ps_a2[bh][:], a2_sb[:, jq:jq + 1],
                                     pair[bh][:], start=(jq == 0),
                                     stop=(jq == JC - 1))

            for j in range(JC):
                wt = load_w8(3, j)
                ps2 = ps_pair("ps_a1_")
                mm8(ps2, wt)
                pair = []
                for bh in range(2):
                    a1b = a1_pool.tile([P, NH], BF16, tag="a1", name="a1b")
                    nc.scalar.activation(a1b[:], ps2[bh][:], AF.Relu,
                                         bias=bias_ap(5, j), scale=1.0 / WS)
                    pair.append(a1b)
                    if dbg and j == 0 and bh == 0:
                        nc.scalar.dma_start(out=dbga1[:, :], in_=a1b[:])
                pend.append((j, pair))
                # defer the tiny a2 matmuls one j so PE never waits on ScalarE
                if len(pend) == 2:
                    flush_a2()
            while pend:
                flush_a2()

            # alpha = sigmoid(a2 @ a1relu + a2_b): [1, BL]; broadcast via
            # DRAM roundtrip that hides under the r1/r2 phases
            for bh in range(2):
                asb = a1_pool.tile([1, NH], F32, tag="a1", name="alpha_sb")
                nc.scalar.activation(asb[:], ps_a2[bh][:], AF.Sigmoid,
                                     bias=bias_sb[0:1, 9 * JC: 9 * JC + 1])
                nc.sync.dma_start(
                    out=alpha_dram[0:1, bh * NH:(bh + 1) * NH], in_=asb[:])
            alpha_rep = consts.tile([P, BL], F32, name="alpha_rep")
            nc.gpsimd.dma_start(
                out=alpha_rep[:], in_=alpha_dram[0:1, :].broadcast_to([P, BL]))
            if dbg:
                nc.scalar.dma_start(out=dbgal[:, :], in_=alpha_rep[:])

            # ---- phase A2: r1 = relu(h @ r1_w.T + b) in fp8 (moving = the
            # resident fp8 h-half). r1 evicts straight to e4m3 pair-tiles
            # so r2 can also run fp8 DoubleRow; r2 evicts bf16 for r3. ----
            r1q = [r1_pool.tile([P, QK, BL], FP8, tag="r1a", name="r1a"),
                   r1_pool.tile([P, QK, BL], FP8, tag="r1b", name="r1b")]
            for j in range(JC):
                wt = w8_pool.tile([P, KC1, P], FP8, tag="w8r", bufs=3,
                                  name=f"w8r1_{j}")
                nc.sync.dma_start(out=wt[:], in_=w8r1[j])
                ps2 = ps_pair("ps_r1_")
                mm8(ps2, wt, tc=KC1 // 2, t0=TC2 // 2)
                qq, jj = divmod(j, QK)
                for bh in range(2):
                    nc.scalar.activation(
                        r1q[qq][:, jj, bh * NH:(bh + 1) * NH],
                        ps2[bh][:], AF.Relu, bias=bias_ap(6, j),
                        scale=1.0 / WS)
                if j == 0:
                    load_bulk_xh16(alpha_rep[0:1, 0:1])
                    load_bulk_cp(alpha_rep[0:1, 0:1])
            if dbg:
                nc.scalar.dma_start(out=dbgr1[:, :], in_=r1q[0][:, 0, :])

            def rhs_r2(k, mv):
                return r2[k][:, mv]

            # ---- phase A3: r2 = relu(r1 @ r2_w.T + b) in fp8 DoubleRow ----
            r2 = []
            for j in range(JC):
                wt = w8_pool.tile([P, KC1, P], FP8, tag="w8r2", bufs=8,
                                  name=f"w8r2_{j}")
                nc.sync.dma_start(out=wt[:], in_=w8r2[j])
                t_ = r2_pool.tile([P, BL], BF16, tag=f"r2_{j}", name=f"r2_{j}")
                ps2 = ps_pair("ps_r2_")
                for t in range(KC1 // 2):
                    for bh in range(2):
                        mv = slice(bh * NH, (bh + 1) * NH)
                        nc.tensor.matmul(
                            ps2[bh][:], wt[:, 2 * t:2 * t + 2, :],
                            r1q[t // 2][:, (2 * t) % QK:(2 * t) % QK + 2, mv],
                            start=(t == 0), stop=(t == KC1 // 2 - 1),
                            perf_mode=DR)
                for bh in range(2):
                    nc.scalar.activation(t_[:, bh * NH:(bh + 1) * NH], ps2[bh][:],
                                         AF.Relu, bias=bias_ap(7, j),
                                         scale=1.0 / WS)
                r2.append(t_)
            if dbg:
                nc.scalar.dma_start(out=dbgr2[:, :], in_=r2[0][:])

            # ---- phase B: gates + r3 + combine, per feature tile j.
            # Order c,s,i (fp8), o, f (bf16), r3: the elementwise chain runs
            # while later matmuls stream; o comes before f/r3 so only the
            # short r3-evict -> add -> tanh -> mul chain trails the last MM.
            GATE8 = {"c": (0, 3, AF.Tanh), "s": (1, 4, AF.Sigmoid),
                     "i": (2, 0, AF.Sigmoid)}

            def gate8(key, j):
                gi, v, fn = GATE8[key]
                wt = load_w8(gi, j)
                t_ = g_pool.tile([P, BL], F32, tag=f"g8{key}", name=f"g8{key}")
                ps2 = ps_pair("ps_g8")
                mm8(ps2, wt)
                for bh in range(2):
                    nc.scalar.activation(t_[:, bh * NH:(bh + 1) * NH],
                                         ps2[bh][:], fn, bias=bias_ap(v, j),
                                         scale=1.0 / WS)
                return t_

            def gate16(gi, v, j):
                wt = load_wg16(gi, j)
                t_ = g_pool.tile([P, BL], F32, tag=f"g16{gi}", name=f"g16{gi}")
                ps2 = ps_pair("ps_g16")
                mm16(ps2, wt, rhs_xh16, KC2)
                for bh in range(2):
                    nc.scalar.activation(t_[:, bh * NH:(bh + 1) * NH],
                                         ps2[bh][:], AF.Sigmoid,
                                         bias=bias_ap(v, j))
                return t_

            for j in range(JC):
                ch = gate8("c", j)
                st = gate8("s", j)
                it = gate8("i", j)

                t1s = []
                for bh in range(2):
                    mv = slice(bh * NH, (bh + 1) * NH)
                    t1 = ew_pool.tile([P, NH], F32, tag=f"t1{bh}", name="t1")
                    nc.vector.tensor_mul(t1[:], it[:, mv], ch[:, mv])
                    nc.vector.tensor_mul(t1[:], t1[:], st[:, mv])
                    nc.vector.tensor_mul(t1[:], t1[:], alpha_rep[:, mv])
                    t1s.append(t1)

                ot = gate16(1, 2, j)

                ft = gate16(0, 1, j)
                for bh in range(2):
                    mv = slice(bh * NH, (bh + 1) * NH)
                    qq, kk = divmod(j, QK)
                    t2 = ew_pool.tile([P, NH], F32, tag=f"t2{bh}", name="t2",
                                      bufs=1)
                    nc.vector.tensor_mul(t2[:], ft[:, mv],
                                         cpq[qq][:, kk, mv])
                    nc.vector.tensor_add(t1s[bh][:], t1s[bh][:], t2[:])

                wt = load_wr16(2, j)
                ps2 = ps_pair("ps_r3_")
                mm16(ps2, wt, rhs_r2, KC1)
                if dbg and j == 0:
                    r3d = g_pool.tile([P, BL], F32, tag="r3d", name="r3d")
                    for bh in range(2):
                        nc.scalar.activation(
                            r3d[:, bh * NH:(bh + 1) * NH], ps2[bh][:],
                            AF.Identity, bias=bias_ap(8, j))
                    for gi, gt in enumerate([ch, st, it, ot, ft, r3d]):
                        nc.scalar.dma_start(out=dbgg[gi], in_=gt[:])
                for bh in range(2):
                    mv = slice(bh * NH, (bh + 1) * NH)
                    # stage[:, 0, :] = h, stage[:, 1, :] = c -> single store.
                    # c = (r3_psum + r3_bias) + t1 in one DVE op straight
                    # from PSUM: no ScalarE eviction on the r3 path.
                    stg = ew_pool.tile([P, 2, NH], BF16, tag=f"st{bh}",
                                       name="stg")
                    nc.vector.scalar_tensor_tensor(
                        stg[:, 1, :], ps2[bh][:], bias_ap(8, j), t1s[bh][:],
                        mybir.AluOpType.add, mybir.AluOpType.add)
                    th = ew_pool.tile([P, NH], F32, tag=f"th{bh}", name="th",
                                      bufs=1)
                    nc.scalar.activation(th[:], stg[:, 1, :], AF.Tanh)
                    nc.vector.tensor_mul(stg[:, 0, :], ot[:, mv], th[:])
                    nc.scalar.dma_start(out=out[j * P:(j + 1) * P, :, mv],
                                        in_=stg[:])

    nc.finalize()
    return nc


def _pack_w(W, kdim):
    # pack[j, p, k*128+m] = W[j*128+m, k*128+p]
    kc = kdim // P
    return np.ascontiguousarray(
        np.asarray(W, np.float32).reshape(JC, P, kc, P)
        .transpose(0, 3, 2, 1).reshape(JC, P, kc * P))


def _pack_act(aT, nq, qk=QK):
    # aT: [nq*qk*P, BL] -> [nq, P, qk, BL] with [q, p, kk, n] = aT[(q*qk+kk)*P+p, n]
    return np.ascontiguousarray(
        aT.reshape(nq, qk, P, BL).transpose(0, 2, 1, 3))


def _prepare(inputs):
    f = lambda name: np.asarray(inputs[name], dtype=np.float32)

    def comb(g):
        u = "U" + g[1]
        return np.concatenate([f(g + "_w"), f(u + "_w")], axis=1)

    # fp8 gates: c, s, i, a1 (order matches in-kernel GATE8/a1 indices)
    w8 = np.stack([
        _pack_w(comb("Wc") * WS, K2),
        _pack_w(comb("Ws") * WS, K2),
        _pack_w(comb("Wi") * WS, K2),
        _pack_w(f("a1_w") * WS, K2),
    ]).astype(E4NP).reshape(4, JC, P, KC2, P)
    w8r1 = (_pack_w(f("r1_w") * WS, D).astype(E4NP)
            .reshape(JC, P, KC1, P))
    w8r2 = (_pack_w(f("r2_w") * WS, D).astype(E4NP)
            .reshape(JC, P, KC1, P))
    # bf16 gates: f, o
    wf_ = np.stack([_pack_w(comb("Wf"), K2),
                    _pack_w(comb("Wo"), K2)]).astype(BFNP)
    wr_ = np.stack(
        [_pack_w(f(n + "_w"), D) for n in ("r1", "r2", "r3")]).astype(BFNP)
    a2p = np.ascontiguousarray(f("a2_w").reshape(KC1, P).T).astype(BFNP)

    bias_vecs = []
    for g in ("Wi", "Wf", "Wo", "Wc", "Ws"):
        u = "U" + g[1]
        bias_vecs.append(f(g + "_b") + f(u + "_b"))
    bias_vecs += [f("a1_b"), f("r1_b"), f("r2_b"), f("r3_b"),
                  np.full(D, f("a2_b")[0], np.float32)]
    # biasp[p, v*JC + j] = vec_v[j*128 + p]
    biasp = np.ascontiguousarray(
        np.stack(bias_vecs).reshape(10, JC, P).transpose(2, 0, 1)
        .reshape(P, 10 * JC))

    x, h, c = f("x"), f("h_prev"), f("c_prev")
    shared = {"w8": w8, "w8r1": w8r1, "w8r2": w8r2, "wf": wf_, "wr": wr_,
              "a2p": a2p, "biasp": biasp}
    in_maps = []
    for core in range(NCORES):
        sl = slice(core * BL, (core + 1) * BL)
        xhT = np.ascontiguousarray(
            np.concatenate([x[sl].T, h[sl].T], axis=0))  # [K2, BL]
        in_maps.append({**shared,
                        "xh16s": _pack_act(xhT.astype(BFNP), 4),
                        "xh8s": _pack_act(xhT.astype(E4NP), 4),
                        "cTs": _pack_act(
                            np.ascontiguousarray(c[sl].T).astype(BFNP), 2)})
    return in_maps


def _run(inputs, trace=False):
    from concourse.bass_utils import run_bass_kernel_spmd

    if "nc" not in _CACHE:
        _CACHE["nc"] = _build()
    nc = _CACHE["nc"]
    in_maps = _prepare(inputs)
    res = run_bass_kernel_spmd(nc, in_maps, core_ids=list(range(NCORES)),
                               trace=trace)
    h = np.empty((B, D), np.float32)
    c = np.empty((B, D), np.float32)
    for core in range(NCORES):
        o = res.results[core]["out"]  # [D, 2, BL] bf16
        sl = slice(core * BL, (core + 1) * BL)
        h[sl] = o[:, 0].T.astype(np.float32)
        c[sl] = o[:, 1].T.astype(np.float32)
    return (h, c), res


def kernel(**inputs):
    (h, c), _ = _run(inputs, trace=False)
    return (h, c)


# revision 82
# speedup vs baseline: 1.2596x; 1.0010x over previous
"""AdaptiveLSTMCellWithRes on 8 TRN2 NeuronCores.

Data-parallel over batch (1024 rows/core), weights replicated.
All on-chip compute happens in transposed-activation space [feat, batch].
Mixed precision:
  - i, s, c_hat, a1 matmuls run fp8(e4m3) with DoubleRow perf mode
    (2 k-tiles per PE pass, 2x the bf16 rate). Weights are pre-scaled
    by 64 on host (0.02-std values would land subnormal in e4m3);
    the 1/64 folds into the PSUM-evicting activation's scale.
  - f, o, residual chain and a2 (the error-critical terms) run bf16.
  - PSUM, biases and the elementwise combine stay fp32; h/c outputs
    are written bf16 (well inside the error budget, halves store DMA).
DMA dispatch costs ~0.6us per dma_start on the issuing engine, so
activations/c_prev/outputs ride the Scalar HWDGE queue in a few big
host-packed transfers while weight slabs stream on the Sync queue.
"""

import sys

if "/opt/trn_rl_repo" not in sys.path:
    sys.path.insert(0, "/opt/trn_rl_repo")

import numpy as np
import ml_dtypes

P = 128
B = 8192          # global batch
NCORES = 8
BL = B // NCORES  # batch per core (1024)
D = 1024          # feature dim
K2 = 2048         # concat(x, h) contraction
JC = D // P       # 8 output-feature tiles
KC2 = K2 // P     # 16 k-chunks for gates/a1
KC1 = D // P      # 8 k-chunks for residual/a2
TC2 = KC2 // 2    # 8 fp8 double-row pair steps
NH = BL // 2      # moving free dim per matmul (512)
WS = 64.0         # fp8 weight pre-scale
QK = 4            # k-chunks per packed activation quarter-tile

E4NP = ml_dtypes.float8_e4m3
BFNP = ml_dtypes.bfloat16

_CACHE = {}


def _build(dbg=False):
    import concourse.bass as bass  # noqa: F401
    from concourse import bacc, mybir
    import concourse.tile as tile

    F32 = mybir.dt.float32
    BF16 = mybir.dt.bfloat16
    FP8 = mybir.dt.float8e4
    AF = mybir.ActivationFunctionType
    DR = mybir.MatmulPerfMode.DoubleRow

    nc = bacc.Bacc()

    # fp8 gates (c, s, i, a1): pack[g, j, p, kk, m] = e4m3(WS * Wg[j*128+m, kk*128+p])
    w8 = nc.declare_dram_parameter("w8", [4, JC, P, KC2, P], FP8, isOutput=False)
    # fp8 r1/r2 weights, same packing with K=D
    w8r1 = nc.declare_dram_parameter("w8r1", [JC, P, KC1, P], FP8,
                                     isOutput=False)
    w8r2 = nc.declare_dram_parameter("w8r2", [JC, P, KC1, P], FP8,
                                     isOutput=False)
    # bf16 gates (f, o): pack[g, j, p, k*128+m] = W[j*128+m, k*128+p]
    wf = nc.declare_dram_parameter("wf", [2, JC, P, K2], BF16, isOutput=False)
    # residual weights (r1, r2, r3): packed [3, JC, P, D] bf16
    wr = nc.declare_dram_parameter("wr", [3, JC, P, D], BF16, isOutput=False)
    # a2 weight: [P, KC1] bf16 with a2p[p, k] = a2_w[0, k*128+p]
    a2p = nc.declare_dram_parameter("a2p", [P, KC1], BF16, isOutput=False)
    # biases: [P, 10*JC]; col v*JC+j holds vec_v[j*128:(j+1)*128]
    # v: 0..4 = combined gate biases (i,f,o,c,s), 5=a1_b, 6=r1_b, 7=r2_b,
    # 8=r3_b, 9=a2_b (replicated)
    biasp = nc.declare_dram_parameter("biasp", [P, 10 * JC], F32, isOutput=False)
    # activations pre-swizzled into quarter tiles: [q, p, kk, n] =
    # act[(q*QK+kk)*128+p, n]; q 0..1 = x^T, 2..3 = h^T
    xh16s = nc.declare_dram_parameter("xh16s", [4, P, QK, BL], BF16,
                                      isOutput=False)
    xh8s = nc.declare_dram_parameter("xh8s", [4, P, QK, BL], FP8,
                                     isOutput=False)
    cTs = nc.declare_dram_parameter("cTs", [2, P, QK, BL], BF16, isOutput=False)
    # out[d, 0, n] = h_t^T, out[d, 1, n] = c_t^T (bf16)
    out = nc.declare_dram_parameter("out", [D, 2, BL], BF16, isOutput=True)

    alpha_dram = nc.dram_tensor("alpha_dram", [1, BL], F32)

    if dbg:
        dbg8 = nc.declare_dram_parameter("dbg8", [P, 2, BL], FP8, isOutput=True)
        dbga1 = nc.declare_dram_parameter("dbga1", [P, NH], BF16, isOutput=True)
        dbgal = nc.declare_dram_parameter("dbgal", [P, BL], F32, isOutput=True)
        dbgr1 = nc.declare_dram_parameter("dbgr1", [P, BL], FP8, isOutput=True)
        dbgr2 = nc.declare_dram_parameter("dbgr2", [P, BL], BF16, isOutput=True)
        dbgg = nc.declare_dram_parameter("dbgg", [6, P, BL], F32, isOutput=True)

    with tile.TileContext(nc) as tc:
        with (
            tc.tile_pool(name="consts", bufs=1) as consts,
            tc.tile_pool(name="xh8", bufs=1) as xh8_pool,
            tc.tile_pool(name="xh16", bufs=1) as xh16_pool,
            tc.tile_pool(name="cpre", bufs=1) as cp_pool,
            tc.tile_pool(name="w8p", bufs=6) as w8_pool,
            tc.tile_pool(name="wfp", bufs=4) as wf_pool,
            tc.tile_pool(name="a1s", bufs=4) as a1_pool,
            tc.tile_pool(name="r1", bufs=1) as r1_pool,
            tc.tile_pool(name="r2", bufs=1) as r2_pool,
            tc.tile_pool(name="gates", bufs=1) as g_pool,
            tc.tile_pool(name="ew", bufs=2) as ew_pool,
            tc.tile_pool(name="psum", bufs=3, space="PSUM") as psum_pool,
            tc.tile_pool(name="psum_a2", bufs=1, space="PSUM") as psum_a2_pool,
        ):
            bias_sb = consts.tile([P, 10 * JC], F32, name="bias_sb")
            a2_sb = consts.tile([P, KC1], BF16, name="a2_sb")

            def bias_ap(v, j):
                return bias_sb[:, v * JC + j: v * JC + j + 1]

            def load_w8(g, j):
                wt = w8_pool.tile([P, KC2, P], FP8, tag="w8", name=f"w8_{g}_{j}")
                nc.sync.dma_start(out=wt[:], in_=w8[g, j])
                return wt

            # ---- prefix. Scalar queue: ONLY the fp8 xh stream (a1/r1 are
            # the first phases) — DMA dispatches share the ScalarE
            # instruction stream with PSUM evictions, so it must stay
            # near-empty. Sync: consts then in-loop weight slabs. GpSimd
            # (idle otherwise): all bulk loads needed only in phase B.
            nc.sync.dma_start(out=bias_sb[:], in_=biasp[:, :])
            nc.sync.dma_start(out=a2_sb[:], in_=a2p[:, :])
            # first quarter split into two pair-tiles: the first matmul's
            # moving data lands sooner on the cold DMA path
            x8a = xh8_pool.tile([P, 2, BL], FP8, tag="x8a", name="x8a")
            nc.scalar.dma_start(out=x8a[:], in_=xh8s[0][:, 0:2, :])
            x8b = xh8_pool.tile([P, 2, BL], FP8, tag="x8b", name="x8b")
            nc.scalar.dma_start(out=x8b[:], in_=xh8s[0][:, 2:4, :])
            xh8q = [None]
            for q in range(1, 4):
                tl = xh8_pool.tile([P, QK, BL], FP8, tag=f"x8{q}", name=f"x8{q}")
                nc.scalar.dma_start(out=tl[:], in_=xh8s[q])
                xh8q.append(tl)
            xh16q = [None] * 4
            cpq = []

            def load_bulk_xh16(dep_ap):
                # phase-B bulk loads on the otherwise-idle gpsimd queue.
                # The scheduler orders DMAs by dependency, not program
                # order, so a tiny write sourced from an early phase-A
                # product (overwritten by the DMA) holds these transfers
                # back until the critical fp8 prefix stream has landed.
                for q in (0, 1, 2, 3):
                    tl = xh16_pool.tile([P, QK, BL], BF16, tag=f"x{q}",
                                        name=f"x{q}")
                    nc.scalar.copy(tl[0:1, 0, 0:1], dep_ap)
                    nc.gpsimd.dma_start(out=tl[:], in_=xh16s[q])
                    xh16q[q] = tl

            def load_bulk_cp(dep_ap):
                for q in range(2):
                    tl = cp_pool.tile([P, QK, BL], BF16, tag=f"cp{q}",
                                      name=f"cp{q}")
                    nc.scalar.copy(tl[0:1, 0, 0:1], dep_ap)
                    nc.gpsimd.dma_start(out=tl[:], in_=cTs[q])
                    cpq.append(tl)

            if dbg:
                nc.scalar.dma_start(out=dbg8[:, :, :], in_=x8a[:, :, :])

            def rhs_xh8(t, mv):
                if t == 0:
                    return x8a[:, :, mv]
                if t == 1:
                    return x8b[:, :, mv]
                q, kk = divmod(2 * t, QK)
                return xh8q[q][:, kk:kk + 2, mv]

            def rhs_xh16(k, mv):
                q, kk = divmod(k, QK)
                return xh16q[q][:, kk:kk + 1, mv]

            def load_wg16(g, j):
                wt = wf_pool.tile([P, K2], BF16, tag="wg", name=f"wg_{g}_{j}",
                                  bufs=3)
                nc.sync.dma_start(out=wt[:], in_=wf[g, j])
                return wt

            def load_wr16(ri, j):
                wt = wf_pool.tile([P, D], BF16, tag="wr", name=f"wr_{ri}_{j}",
                                  bufs=6)
                nc.sync.dma_start(out=wt[:], in_=wr[ri, j])
                return wt

            def ps_pair(nm):
                return [psum_pool.tile([P, NH], F32, tag="ps0", name=f"{nm}0"),
                        psum_pool.tile([P, NH], F32, tag="ps1", name=f"{nm}1")]

            def mm8(ps2, wt, tc=TC2, t0=0):
                # fp8 DoubleRow over tc k-pairs starting at pair t0; each
                # stationary pair feeds both batch halves
                for t in range(tc):
                    for bh in range(2):
                        mv = slice(bh * NH, (bh + 1) * NH)
                        nc.tensor.matmul(
                            ps2[bh][:], wt[:, 2 * t:2 * t + 2, :],
                            rhs_xh8(t0 + t, mv),
                            start=(t == 0), stop=(t == tc - 1),
                            perf_mode=DR)

            def mm16(ps2, wt, rhs, kc, koff=0):
                # k outer / bh inner: each stationary tile feeds 2 matmuls
                for k in range(kc):
                    for bh in range(2):
                        mv = slice(bh * NH, (bh + 1) * NH)
                        nc.tensor.matmul(
                            ps2[bh][:], wt[:, k * P:(k + 1) * P],
                            rhs(koff + k, mv),
                            start=(k == 0), stop=(k == kc - 1))

            # ---- phase A1: a1 (fp8), deferred tiny a2 matmuls ----
            ps_a2 = [psum_a2_pool.tile([1, NH], F32, tag="a20", name="psa20"),
                     psum_a2_pool.tile([1, NH], F32, tag="a21", name="psa21")]
            pend = []

            def flush_a2():
                jq, pair = pend.pop(0)
                for bh in range(2):
                    nc.tensor.matmul(ps_a2[bh][:], a2_sb[:, jq:jq + 1],
                                     pair[bh][:], start=(jq == 0),
                                     stop=(jq == JC - 1))

            for j in range(JC):
                wt = load_w8(3, j)
                ps2 = ps_pair("ps_a1_")
                mm8(ps2, wt)
                pair = []
                for bh in range(2):
                    a1b = a1_pool.tile([P, NH], BF16, tag="a1", name="a1b")
                    nc.scalar.activation(a1b[:], ps2[bh][:], AF.Relu,
                                         bias=bias_ap(5, j), scale=1.0 / WS)
                    pair.append(a1b)
                    if dbg and j == 0 and bh == 0:
                        nc.scalar.dma_start(out=dbga1[:, :], in_=a1b[:])
                pend.append((j, pair))
                # defer the tiny a2 matmuls one j so PE never waits on ScalarE
                if len(pend) == 2:
                    flush_a2()
            while pend:
                flush_a2()

            # alpha = sigmoid(a2 @ a1relu + a2_b): [1, BL]; broadcast via
            # DRAM roundtrip that hides under the r1/r2 phases
            for bh in range(2):
                asb = a1_pool.tile([1, NH], F32, tag="a1", name="alpha_sb")
                nc.scalar.activation(asb[:], ps_a2[bh][:], AF.Sigmoid,
                                     bias=bias_sb[0:1, 9 * JC: 9 * JC + 1])
                nc.sync.dma_start(
                    out=alpha_dram[0:1, bh * NH:(bh + 1) * NH], in_=asb[:])
            alpha_rep = consts.tile([P, BL], F32, name="alpha_rep")
            nc.gpsimd.dma_start(
                out=alpha_rep[:], in_=alpha_dram[0:1, :].broadcast_to([P, BL]))
            if dbg:
                nc.scalar.dma_start(out=dbgal[:, :], in_=alpha_rep[:])

            # ---- phase A2: r1 = relu(h @ r1_w.T + b) in fp8 (moving = the
            # resident fp8 h-half). r1 evicts straight to e4m3 pair-tiles
            # so r2 can also run fp8 DoubleRow; r2 evicts bf16 for r3. ----
            r1q = [r1_pool.tile([P, QK, BL], FP8, tag="r1a", name="r1a"),
                   r1_pool.tile([P, QK, BL], FP8, tag="r1b", name="r1b")]
            for j in range(JC):
                wt = w8_pool.tile([P, KC1, P], FP8, tag="w8r", bufs=3,
                                  name=f"w8r1_{j}")
                nc.sync.dma_start(out=wt[:], in_=w8r1[j])
                ps2 = ps_pair("ps_r1_")
                mm8(ps2, wt, tc=KC1 // 2, t0=TC2 // 2)
                qq, jj = divmod(j, QK)
                for bh in range(2):
                    nc.scalar.activation(
                        r1q[qq][:, jj, bh * NH:(bh + 1) * NH],
                        ps2[bh][:], AF.Relu, bias=bias_ap(6, j),
                        scale=1.0 / WS)
                if j == 0:
                    load_bulk_xh16(alpha_rep[0:1, 0:1])
                    load_bulk_cp(alpha_rep[0:1, 0:1])
            if dbg:
                nc.scalar.dma_start(out=dbgr1[:, :], in_=r1q[0][:, 0, :])

            def rhs_r2(k, mv):
                return r2[k][:, mv]

            # ---- phase A3: r2 = relu(r1 @ r2_w.T + b) in fp8 DoubleRow ----
            r2 = []
            for j in range(JC):
                wt = w8_pool.tile([P, KC1, P], FP8, tag="w8r2", bufs=8,
                                  name=f"w8r2_{j}")
                nc.sync.dma_start(out=wt[:], in_=w8r2[j])
                t_ = r2_pool.tile([P, BL], BF16, tag=f"r2_{j}", name=f"r2_{j}")
                ps2 = ps_pair("ps_r2_")
                for t in range(KC1 // 2):
                    for bh in range(2):
                        mv = slice(bh * NH, (bh + 1) * NH)
                        nc.tensor.matmul(
                            ps2[bh][:], wt[:, 2 * t:2 * t + 2, :],
                            r1q[t // 2][:, (2 * t) % QK:(2 * t) % QK + 2, mv],
                            start=(t == 0), stop=(t == KC1 // 2 - 1),
                            perf_mode=DR)
                for bh in range(2):
                    nc.scalar.activation(t_[:, bh * NH:(bh + 1) * NH], ps2[bh][:],
                                         AF.Relu, bias=bias_ap(7, j),
                                         scale=1.0 / WS)
                r2.append(t_)
            if dbg:
                nc.scalar.dma_start(out=dbgr2[:, :], in_=r2[0][:])

            # ---- phase B: gates + r3 + combine, per feature tile j.
            # Order c,s,i (fp8), o, f (bf16), r3: the elementwise chain runs
            # while later matmuls stream; o comes before f/r3 so only the
            # short r3-evict -> add -> tanh -> mul chain trails the last MM.
            GATE8 = {"c": (0, 3, AF.Tanh), "s": (1, 4, AF.Sigmoid),
                     "i": (2, 0, AF.Sigmoid)}

            def gate8(key, j):
                gi, v, fn = GATE8[key]
                wt = load_w8(gi, j)
                t_ = g_pool.tile([P, BL], F32, tag=f"g8{key}", name=f"g8{key}")
                ps2 = ps_pair("ps_g8")
                mm8(ps2, wt)
                for bh in range(2):
                    nc.scalar.activation(t_[:, bh * NH:(bh + 1) * NH],
                                         ps2[bh][:], fn, bias=bias_ap(v, j),
                                         scale=1.0 / WS)
                return t_

            def gate16(gi, v, j):
                wt = load_wg16(gi, j)
                t_ = g_pool.tile([P, BL], F32, tag=f"g16{gi}", name=f"g16{gi}")
                ps2 = ps_pair("ps_g16")
                mm16(ps2, wt, rhs_xh16, KC2)
                for bh in range(2):
                    nc.scalar.activation(t_[:, bh * NH:(bh + 1) * NH],
                                         ps2[bh][:], AF.Sigmoid,
                                         bias=bias_ap(v, j))
                return t_

            for j in range(JC):
                ch = gate8("c", j)
                st = gate8("s", j)
                it = gate8("i", j)

                t1s = []
                for bh in range(2):
                    mv = slice(bh * NH, (bh + 1) * NH)
                    t1 = ew_pool.tile([P, NH], F32, tag=f"t1{bh}", name="t1")
                    nc.vector.tensor_mul(t1[:], it[:, mv], ch[:, mv])
                    nc.vector.tensor_mul(t1[:], t1[:], st[:, mv])
                    nc.vector.tensor_mul(t1[:], t1[:], alpha_rep[:, mv])
                    t1s.append(t1)

                ot = gate16(1, 2, j)

                ft = gate16(0, 1, j)
                for bh in range(2):
                    mv = slice(bh * NH, (bh + 1) * NH)
                    qq, kk = divmod(j, QK)
                    t2 = ew_pool.tile([P, NH], F32, tag=f"t2{bh}", name="t2",
                                      bufs=1)
                    nc.vector.tensor_mul(t2[:], ft[:, mv],
                                         cpq[qq][:, kk, mv])
                    nc.vector.tensor_add(t1s[bh][:], t1s[bh][:], t2[:])

                wt = load_wr16(2, j)
                ps2 = ps_pair("ps_r3_")
                if j < JC - 1:
                    mm16(ps2, wt, rhs_r2, KC1)
                else:
                    # last j: bh-outer so ps2[0] completes a half-phase
                    # early and its finish chain overlaps ps2[1]'s matmuls
                    for bh in range(2):
                        mv = slice(bh * NH, (bh + 1) * NH)
                        for k in range(KC1):
                            nc.tensor.matmul(
                                ps2[bh][:], wt[:, k * P:(k + 1) * P],
                                rhs_r2(k, mv),
                                start=(k == 0), stop=(k == KC1 - 1))
                if dbg and j == 0:
                    r3d = g_pool.tile([P, BL], F32, tag="r3d", name="r3d")
                    for bh in range(2):
                        nc.scalar.activation(
                            r3d[:, bh * NH:(bh + 1) * NH], ps2[bh][:],
                            AF.Identity, bias=bias_ap(8, j))
                    for gi, gt in enumerate([ch, st, it, ot, ft, r3d]):
                        nc.scalar.dma_start(out=dbgg[gi], in_=gt[:])
                # stage[:, 0, :] = h, stage[:, 1, :] = c -> single store.
                # c = (r3_psum + r3_bias) + t1 in one DVE op straight
                # from PSUM: no ScalarE eviction on the r3 path. The last
                # j is processed in NH/2 chunks to pipeline the trailing
                # add -> tanh -> mul chain across Vector/Scalar.
                nch = 1 if j < JC - 1 else 2
                hw_ = NH // nch
                for bh in range(2):
                    for cc in range(nch):
                        pv = slice(cc * hw_, (cc + 1) * hw_)
                        mv = slice(bh * NH + cc * hw_,
                                   bh * NH + (cc + 1) * hw_)
                        stg = ew_pool.tile([P, 2, hw_], BF16,
                                           tag=f"st{bh}_{nch}{cc}", name="stg")
                        nc.vector.scalar_tensor_tensor(
                            stg[:, 1, :], ps2[bh][:, pv], bias_ap(8, j),
                            t1s[bh][:, pv],
                            mybir.AluOpType.add, mybir.AluOpType.add)
                        th = ew_pool.tile([P, hw_], F32, tag=f"th{bh}{cc}",
                                          name="th", bufs=1)
                        nc.scalar.activation(th[:], stg[:, 1, :], AF.Tanh)
                        nc.vector.tensor_mul(stg[:, 0, :], ot[:, mv], th[:])
                        nc.scalar.dma_start(
                            out=out[j * P:(j + 1) * P, :, mv], in_=stg[:])

    nc.finalize()
    return nc


def _pack_w(W, kdim):
    # pack[j, p, k*128+m] = W[j*128+m, k*128+p]
    kc = kdim // P
    return np.ascontiguousarray(
        np.asarray(W, np.float32).reshape(JC, P, kc, P)
        .transpose(0, 3, 2, 1).reshape(JC, P, kc * P))


def _pack_act(aT, nq, qk=QK):
    # aT: [nq*qk*P, BL] -> [nq, P, qk, BL] with [q, p, kk, n] = aT[(q*qk+kk)*P+p, n]
    return np.ascontiguousarray(
        aT.reshape(nq, qk, P, BL).transpose(0, 2, 1, 3))


def _prepare(inputs):
    f = lambda name: np.asarray(inputs[name], dtype=np.float32)

    def comb(g):
        u = "U" + g[1]
        return np.concatenate([f(g + "_w"), f(u + "_w")], axis=1)

    # fp8 gates: c, s, i, a1 (order matches in-kernel GATE8/a1 indices)
    w8 = np.stack([
        _pack_w(comb("Wc") * WS, K2),
        _pack_w(comb("Ws") * WS, K2),
        _pack_w(comb("Wi") * WS, K2),
        _pack_w(f("a1_w") * WS, K2),
    ]).astype(E4NP).reshape(4, JC, P, KC2, P)
    w8r1 = (_pack_w(f("r1_w") * WS, D).astype(E4NP)
            .reshape(JC, P, KC1, P))
    w8r2 = (_pack_w(f("r2_w") * WS, D).astype(E4NP)
            .reshape(JC, P, KC1, P))
    # bf16 gates: f, o
    wf_ = np.stack([_pack_w(comb("Wf"), K2),
                    _pack_w(comb("Wo"), K2)]).astype(BFNP)
    wr_ = np.stack(
        [_pack_w(f(n + "_w"), D) for n in ("r1", "r2", "r3")]).astype(BFNP)
    a2p = np.ascontiguousarray(f("a2_w").reshape(KC1, P).T).astype(BFNP)

    bias_vecs = []
    for g in ("Wi", "Wf", "Wo", "Wc", "Ws"):
        u = "U" + g[1]
        bias_vecs.append(f(g + "_b") + f(u + "_b"))
    bias_vecs += [f("a1_b"), f("r1_b"), f("r2_b"), f("r3_b"),
                  np.full(D, f("a2_b")[0], np.float32)]
    # biasp[p, v*JC + j] = vec_v[j*128 + p]
    biasp = np.ascontiguousarray(
        np.stack(bias_vecs).reshape(10, JC, P).transpose(2, 0, 1)
        .reshape(P, 10 * JC))

    x, h, c = f("x"), f("h_prev"), f("c_prev")
    shared = {"w8": w8, "w8r1": w8r1, "w8r2": w8r2, "wf": wf_, "wr": wr_,
              "a2p": a2p, "biasp": biasp}
    in_maps = []
    for core in range(NCORES):
        sl = slice(core * BL, (core + 1) * BL)
        xhT = np.ascontiguousarray(
            np.concatenate([x[sl].T, h[sl].T], axis=0))  # [K2, BL]
        in_maps.append({**shared,
                        "xh16s": _pack_act(xhT.astype(BFNP), 4),
                        "xh8s": _pack_act(xhT.astype(E4NP), 4),
                        "cTs": _pack_act(
                            np.ascontiguousarray(c[sl].T).astype(BFNP), 2)})
    return in_maps


def _run(inputs, trace=False):
    from concourse.bass_utils import run_bass_kernel_spmd

    if "nc" not in _CACHE:
        _CACHE["nc"] = _build()
    nc = _CACHE["nc"]
    in_maps = _prepare(inputs)
    res = run_bass_kernel_spmd(nc, in_maps, core_ids=list(range(NCORES)),
                               trace=trace)
    h = np.empty((B, D), np.float32)
    c = np.empty((B, D), np.float32)
    for core in range(NCORES):
        o = res.results[core]["out"]  # [D, 2, BL] bf16
        sl = slice(core * BL, (core + 1) * BL)
        h[sl] = o[:, 0].T.astype(np.float32)
        c[sl] = o[:, 1].T.astype(np.float32)
    return (h, c), res


def kernel(**inputs):
    (h, c), _ = _run(inputs, trace=False)
    return (h, c)
